# revision 33
# baseline (speedup 1.0000x reference)
"""3-layer GraphSAGE (mean aggregation) on 8 TRN2 NeuronCores.

Self-contained: hardcoded problem shapes (N=50000, E=800000, H=256, 3 layers).

Strategy
--------
Host side (numpy, inside kernel()):
  * degree-balanced assignment of nodes to 8 cores x 49 blocks of <=128 nodes
  * permuted "table" layout: table row = core*6272 + block*128 + pos
  * blocks processed in GROUPS of GS=7 (uniform 7 groups); per-group edge
    lists as int16 gather indices (lo/hi split at row 32768 for int16 range),
    padded to uniform subtile counts so all 8 cores run one SPMD program
  * the segment-sum one-hot matrix (edge -> within-block position) is
    layer-invariant, so it is built on the HOST in fp8 and passed as input
  * ELU computed as max(z+1, exp(min(z+1,1)-1)) - 1; the +1 and the
    mean-shift correction for storing h+1 between layers are folded into
    the bias vectors (valid because every node has degree >= 1)

Device side (Bass/Tile), per layer:
  gather messages (fp8, 32-subtile calls via 4096-desc SWDGE ring) ->
  segmented sum via fp8 TensorE matmuls against the preloaded one-hot ->
  1/deg scale on Activation engine -> PE transposes -> dense bf16 matmuls
  against W^T (+ K=1 bias matmul) -> shifted ELU (Act copy/exp + DVE
  min/max on bf16) -> group-batched shard writes (bf16 own + fp8 table) ->
  AllGather (fp8, Shared output) between layers.
"""

import os
import sys

sys.path.insert(0, "/opt/trn_rl_repo")

import numpy as np
import ml_dtypes

from concourse import bacc, bass, mybir, tile
from concourse.bass_utils import run_bass_kernel_spmd
from concourse.masks import make_identity

bf16 = ml_dtypes.bfloat16
f8 = ml_dtypes.float8_e4m3

N_NODES = 50000
N_EDGES = 800000
H = 256
NC = 8
P = 128
NB = 49                      # blocks per core
HI_BASE = 32768              # int16 index split point

# knobs (env-overridable for experiments; defaults are the shipping config)
GS = int(os.environ.get("GCN_GS", "7"))          # blocks per gather group
TAB8 = os.environ.get("GCN_TAB8", "0") == "1"    # fp8 h-tables + messages
XTAB8 = os.environ.get("GCN_XTAB8",
                       os.environ.get("GCN_TAB8", "0")) == "1"  # fp8 x table
OH8 = os.environ.get("GCN_OH8", "1") == "1"      # fp8 one-hot (0/1 exact)
HOST_OH = os.environ.get("GCN_HOST_OH", "1") == "1"  # host-built one-hot
OH_RES = os.environ.get("GCN_OH_RES", "auto")    # SBUF-resident lo one-hot:
# "auto" = only when messages are fp8 (bf16 messages + resident oh overflow)
ACT_ELU = os.environ.get("GCN_ACT_ELU", "1") == "1"  # ELU via Act engine
DR = os.environ.get("GCN_DR", "1") == "1"        # DoubleRow on fp8 layers
CC_SHARED = os.environ.get("GCN_CC_SHARED", "1") == "1"
GMAX = int(os.environ.get("GCN_GMAX", "8"))      # subtiles per gather call
DMA_SCRATCH = int(os.environ.get("GCN_SCRATCH", "16384"))  # SWDGE ring bytes
# NOTE: dynamic_dma_scratch_size is charged PER PARTITION in SBUF, and rings
# bigger than the default 16384 (1024-desc) / gather calls above 1024 indices
# crash real HW (NRT_EXEC_UNIT_UNRECOVERABLE) — keep 16384/GMAX=8.
N_LAYERS = int(os.environ.get("GCN_LAYERS", "3"))
USE_CC = os.environ.get("GCN_CC", "1") == "1"

NG = (NB + GS - 1) // GS     # groups per core
STRIDE = NB * P              # 6272 table rows per core
TAB = NC * STRIDE            # 50176 table rows


def _group_blocks(g: int) -> list:
    return list(range(g * GS, min((g + 1) * GS, NB)))


def _assign_blocks(deg: np.ndarray) -> np.ndarray:
    """Serpentine deal of nodes (sorted by degree desc) into NC*NB blocks."""
    nb_total = NC * NB
    order = np.argsort(-deg, kind="stable")
    block_of_node = np.empty(N_NODES, dtype=np.int64)
    pos = 0
    rnd = 0
    while pos < N_NODES:
        take = min(nb_total, N_NODES - pos)
        blocks = np.arange(nb_total) if rnd % 2 == 0 else np.arange(nb_total)[::-1]
        block_of_node[order[pos:pos + take]] = blocks[:take]
        pos += take
        rnd += 1
    return block_of_node


def _preprocess(edge_index: np.ndarray):
    """Graph preprocessing. Returns dict of host-side structures."""
    src = np.asarray(edge_index[0], dtype=np.int64)
    dst = np.asarray(edge_index[1], dtype=np.int64)
    deg = np.bincount(dst, minlength=N_NODES).astype(np.int64)
    shift_ok = bool(deg.min() >= 1)

    block_of_node = _assign_blocks(deg)

    # position of each node within its block; table row of each node
    order = np.lexsort((np.arange(N_NODES), block_of_node))
    pos_in_block = np.empty(N_NODES, dtype=np.int64)
    counts = np.zeros(NC * NB, dtype=np.int64)
    for n in order:
        b = block_of_node[n]
        pos_in_block[n] = counts[b]
        counts[b] += 1
    assert counts.max() <= P, f"block overflow: {counts.max()}"
    table_row = block_of_node * P + pos_in_block

    # edges grouped by destination block
    e_block = block_of_node[dst]
    e_seg = pos_in_block[dst]
    e_srcrow = table_row[src]

    sort_idx = np.argsort(e_block, kind="stable")
    e_block_s = e_block[sort_idx]
    e_seg_s = e_seg[sort_idx]
    e_srcrow_s = e_srcrow[sort_idx]
    blk_starts = np.searchsorted(e_block_s, np.arange(NC * NB + 1))

    lo_counts = np.empty(NC * NB, dtype=np.int64)
    hi_counts = np.empty(NC * NB, dtype=np.int64)
    for b in range(NC * NB):
        rows = e_srcrow_s[blk_starts[b]:blk_starts[b + 1]]
        lo_counts[b] = int((rows < HI_BASE).sum())
        hi_counts[b] = rows.shape[0] - lo_counts[b]
    sub_lo = int(np.ceil(lo_counts.max() / P))
    sub_hi = int(np.ceil(hi_counts.max() / P))
    st = sub_lo + sub_hi

    # per-core packed arrays, group layout:
    #   subtile order per group: [lo(b0)..lo(bN)][hi(b0)..hi(bN)]
    gw = GS * st                                  # subtiles per (full) group
    idx_all = np.zeros((NC, P, NG * gw * 8), dtype=np.int16)
    seg_all = np.full((NC, P, NG * gw), 200.0, dtype=np.float32)
    recip_all = np.zeros((NC, P, NB), dtype=np.float32)

    recip = (1.0 / np.maximum(deg, 1)).astype(np.float32)

    def pack16(flat: np.ndarray) -> np.ndarray:
        # dma_gather layout: unwrapped[k] = tile16[k % 16, k // 16]
        n = flat.shape[0]
        t = flat.reshape(n // 16, 16).T
        return np.tile(t, (8, 1))  # [128, n/16]

    def padded(rows, segs, nsub):
        r = np.zeros(nsub * P, dtype=np.int16)
        r[:rows.shape[0]] = rows.astype(np.int16)
        s = np.full(nsub * P, 200.0, dtype=np.float32)
        s[:segs.shape[0]] = segs.astype(np.float32)
        return r, s.reshape(nsub, P).T  # seg -> [P, nsub]

    for c in range(NC):
        for g in range(NG):
            blocks = _group_blocks(g)
            los, his = [], []
            for lb in blocks:
                b = c * NB + lb
                rows = e_srcrow_s[blk_starts[b]:blk_starts[b + 1]]
                segs = e_seg_s[blk_starts[b]:blk_starts[b + 1]]
                is_lo = rows < HI_BASE
                los.append(padded(rows[is_lo], segs[is_lo], sub_lo))
                his.append(padded(rows[~is_lo] - HI_BASE, segs[~is_lo], sub_hi))

            ng = len(blocks)
            ibase = g * gw * 8
            sbase = g * gw
            lo_flat = np.concatenate([r for r, _ in los])
            hi_flat = np.concatenate([r for r, _ in his])
            idx_all[c, :, ibase:ibase + ng * sub_lo * 8] = pack16(lo_flat)
            idx_all[c, :, ibase + ng * sub_lo * 8:
                    ibase + ng * st * 8] = pack16(hi_flat)
            seg_all[c, :, sbase:sbase + ng * sub_lo] = np.concatenate(
                [s for _, s in los], axis=1)
            seg_all[c, :, sbase + ng * sub_lo:sbase + ng * st] = np.concatenate(
                [s for _, s in his], axis=1)

            for lb in blocks:
                b = c * NB + lb
                nodes_here = np.where(block_of_node == b)[0]
                recip_all[c, pos_in_block[nodes_here], lb] = recip[nodes_here]

    out = dict(
        table_row=table_row, sub_lo=sub_lo, sub_hi=sub_hi, st=st,
        idx_all=idx_all, seg_all=seg_all.astype(bf16), recip_all=recip_all,
        shift_ok=shift_ok,
    )

    if HOST_OH:
        # host-built one-hot: oh[c, p, j, s] = (seg(edge p of subtile j) == s)
        # in fp8 (0/1 exact), split into lo/hi subtile parts per group
        oh = (seg_all[..., None] == np.arange(P, dtype=np.float32)) \
            .astype(f8 if OH8 else bf16)
        oh = oh.reshape(NC, P, NG, gw, P)
        SL, SH = GS * sub_lo, GS * sub_hi
        oh_lo = np.zeros((NC, P, NG, SL, P), dtype=oh.dtype)
        oh_hi = np.zeros((NC, P, NG, SH, P), dtype=oh.dtype)
        for g in range(NG):
            ng = len(_group_blocks(g))
            nlo, nhi = ng * sub_lo, ng * sub_hi
            oh_lo[:, :, g, :nlo] = oh[:, :, g, :nlo]
            oh_hi[:, :, g, :nhi] = oh[:, :, g, nlo:nlo + nhi]
        out["oh_lo_all"] = oh_lo.reshape(NC, P, NG * SL * P)
        out["oh_hi_all"] = oh_hi.reshape(NC, P, NG * SH * P)
    return out


def _group_subtiles(i: int, ng: int, sub_lo: int, sub_hi: int) -> list:
    """Subtile columns of block i (0-based within group) in a group of ng."""
    lo = list(range(i * sub_lo, (i + 1) * sub_lo))
    hi = [ng * sub_lo + i * sub_hi + j for j in range(sub_hi)]
    return lo + hi


def _build(sub_lo: int, sub_hi: int, shift: bool,
           use_cc: bool | None = None,
           cc_shared: bool | None = None, n_layers: int | None = None,
           gmax: int | None = None, scratch: int | None = None,
           tab8: bool | None = None, xtab8: bool | None = None,
           oh8: bool | None = None, dr: bool | None = None,
           host_oh: bool | None = None,
           oh_res: bool | None = None, act_elu: bool | None = None):
    """Build the SPMD Bass program. Returns compiled nc."""
    USE_CC_ = USE_CC if use_cc is None else use_cc
    CC_SHARED_ = CC_SHARED if cc_shared is None else cc_shared
    N_LAYERS_ = N_LAYERS if n_layers is None else n_layers
    GMAX_ = GMAX if gmax is None else gmax
    SCRATCH_ = DMA_SCRATCH if scratch is None else scratch
    TAB8_ = TAB8 if tab8 is None else tab8
    XTAB8_ = XTAB8 if xtab8 is None else xtab8
    OH8_ = OH8 if oh8 is None else oh8
    DR_ = (DR if dr is None else dr) and OH8_
    HOST_OH_ = HOST_OH if host_oh is None else host_oh
    if oh_res is None:
        oh_res = (TAB8_ and XTAB8_) if OH_RES == "auto" else OH_RES == "1"
    OH_RES_ = oh_res and HOST_OH_
    ACT_ELU_ = ACT_ELU if act_elu is None else act_elu

    st = sub_lo + sub_hi
    gw = GS * st
    SL = GS * sub_lo            # lo subtiles per full group
    SH = GS * sub_hi            # hi subtiles per full group
    f32 = mybir.dt.float32
    b16 = mybir.dt.bfloat16
    fp8d = mybir.dt.float8e4
    xdt = fp8d if XTAB8_ else b16    # layer-0 table (x) dtype
    hdt = fp8d if TAB8_ else b16     # inter-layer table (h) dtype
    ohdt = fp8d if OH8_ else b16
    assert xdt == hdt, "per-layer message dtypes not supported yet"
    tabdt = xdt
    AT = mybir.ActivationFunctionType

    nc = bacc.Bacc("TRN2", target_bir_lowering=False, debug=False,
                   enable_asserts=True, num_devices=NC,
                   dynamic_dma_scratch_size=SCRATCH_)

    xtab_d = nc.dram_tensor("xtab", [TAB, H], tabdt, kind="ExternalInput")
    xown_d = nc.dram_tensor("xown", [STRIDE, H], b16, kind="ExternalInput")
    wts_d = nc.dram_tensor("wts", [12 * P, H], b16, kind="ExternalInput")
    bias_d = nc.dram_tensor("bias", [3, H], b16, kind="ExternalInput")
    idx_d = nc.dram_tensor("idxall", [P, NG * gw * 8], mybir.dt.int16,
                           kind="ExternalInput")
    recip_d = nc.dram_tensor("recipall", [P, NB], f32, kind="ExternalInput")
    if HOST_OH_:
        ohlo_d = nc.dram_tensor("ohloall", [P, NG * SL * P], ohdt,
                                kind="ExternalInput")
        ohhi_d = nc.dram_tensor("ohhiall", [P, NG * SH * P], ohdt,
                                kind="ExternalInput")
    else:
        seg_d = nc.dram_tensor("segall", [P, NG * gw], b16,
                               kind="ExternalInput")
        iota_d = nc.dram_tensor("iotarep", [P, gw * P], b16,
                                kind="ExternalInput")

    out_d = nc.dram_tensor("out", [STRIDE, H], f32, kind="ExternalOutput")

    with tile.TileContext(nc) as tc:
        with (
            tc.tile_pool(name="const", bufs=1) as cp,
            tc.tile_pool(name="mlo", bufs=2) as mlo,
            tc.tile_pool(name="mhi", bufs=1 if OH_RES_ else 2) as mhi,
            tc.tile_pool(name="ohp", bufs=2) as ohp,
            tc.tile_pool(name="idxp", bufs=2) as idxp,
            tc.tile_pool(name="xgp", bufs=2) as xgp,
            tc.tile_pool(name="hgp", bufs=2) as hgp,
            tc.tile_pool(name="actp", bufs=2) as actp,
            tc.tile_pool(name="elup", bufs=2) as elup,
            tc.tile_pool(name="pa", bufs=3, space="PSUM") as pa,
            tc.tile_pool(name="po", bufs=2, space="PSUM") as po,
            tc.tile_pool(name="ptr", bufs=2, space="PSUM") as ptr,
            tc.tile_pool(name="dram", bufs=1, space="DRAM") as dr,
        ):
            # ---- resident constants ----
            recip_t = cp.tile([P, NB], f32)
            nc.sync.dma_start(out=recip_t[:], in_=recip_d[:])
            if HOST_OH_ and OH_RES_:
                # SBUF-resident lo one-hot (layer-invariant), loaded once.
                ohlo_t = cp.tile([P, NG * SL * P], ohdt)
                for g in range(NG):
                    nc.scalar.dma_start(
                        out=ohlo_t[:, g * SL * P:(g + 1) * SL * P],
                        in_=ohlo_d[:, g * SL * P:(g + 1) * SL * P])
            if not HOST_OH_:
                seg_t = cp.tile([P, NG * gw], b16)
                nc.sync.dma_start(out=seg_t[:], in_=seg_d[:])
                iota_t = cp.tile([P, gw * P], b16)
                nc.sync.dma_start(out=iota_t[:], in_=iota_d[:])
            wts_t = cp.tile([P, 12, H], b16)
            nc.sync.dma_start(
                out=wts_t[:], in_=wts_d[:].rearrange("(c k) h -> k c h", k=P)
            )
            bias_t = cp.tile([1, 3, H], b16)
            nc.sync.dma_start(
                out=bias_t[:], in_=bias_d[:].rearrange("(a c) h -> a c h", a=1)
            )
            ones_t = cp.tile([1, P], b16)
            nc.vector.memset(ones_t[:], 1.0)
            ident_t = cp.tile([P, P], b16)
            make_identity(nc, ident_t[:])

            # internal DRAM for inter-layer activations
            addr_space = "Shared" if CC_SHARED_ else "Local"
            h_own = [
                dr.tile([STRIDE, H], b16, tag=f"h_own{i}", name=f"h_own{i}")
                for i in range(2)
            ]
            h_tab = [
                dr.tile([STRIDE, H], tabdt, tag=f"h_tab{i}", name=f"h_tab{i}")
                for i in range(2)
            ] if TAB8_ else h_own
            h_full = [
                dr.tile([TAB, H], tabdt, tag=f"h_full{i}", name=f"h_full{i}",
                        addr_space=addr_space)
                for i in range(2)
            ]

            for layer in range(N_LAYERS_):
                last = layer == N_LAYERS_ - 1
                if layer == 0:
                    src_tab = xtab_d
                    own = xown_d
                else:
                    src_tab = h_full[layer - 1] if USE_CC_ else xtab_d
                    own = h_own[layer - 1]
                tab_lo = src_tab[:HI_BASE, :]
                tab_hi = src_tab[HI_BASE:, :]

                for g in range(NG):
                    blocks = _group_blocks(g)
                    ng = len(blocks)
                    nsub = ng * st
                    nlo = ng * sub_lo
                    nhi = ng * sub_hi

                    # ---- per-group gather indices ----
                    idx_t = idxp.tile([P, gw * 8], mybir.dt.int16, tag="idx")
                    nc.scalar.dma_start(
                        out=idx_t[:, 0:nsub * 8],
                        in_=idx_d[:, g * gw * 8:(g * gw + nsub) * 8])

                    # ---- gather messages (lo/hi into separate tiles) ----
                    m_lo = mlo.tile([P, SL, H], tabdt, tag="mlo")
                    m_hi = mhi.tile([P, SH, H], tabdt, tag="mhi")
                    for tab, mt, s0, s1 in ((tab_lo, m_lo, 0, nlo),
                                            (tab_hi, m_hi, nlo, nsub)):
                        step = (s1 - s0) if GMAX_ <= 0 else GMAX_
                        for g0 in range(s0, s1, step):
                            g1 = min(g0 + step, s1)
                            nc.gpsimd.dma_gather(
                                mt[:, g0 - s0:g1 - s0, :], tab,
                                idx_t[:, g0 * 8:g1 * 8],
                                (g1 - g0) * P, (g1 - g0) * P, H,
                                single_packet=True,
                            )

                    # ---- one-hot: lo resident (or built), hi streamed ----
                    if HOST_OH_ and OH_RES_:
                        oh_lo_src, oh_lo_base = ohlo_t, g * SL * P
                        oh_hi_src = ohp.tile([P, SH * P], ohdt, tag="onehot")
                        nc.scalar.dma_start(
                            out=oh_hi_src[:, 0:nhi * P],
                            in_=ohhi_d[:, g * SH * P:(g * SH + nhi) * P],
                        )
                        oh_hi_base = 0
                    elif HOST_OH_:
                        oh_lo_src = ohp.tile([P, gw * P], ohdt, tag="onehot")
                        nc.scalar.dma_start(
                            out=oh_lo_src[:, 0:nlo * P],
                            in_=ohlo_d[:, g * SL * P:(g * SL + nlo) * P],
                        )
                        nc.scalar.dma_start(
                            out=oh_lo_src[:, nlo * P:nsub * P],
                            in_=ohhi_d[:, g * SH * P:(g * SH + nhi) * P],
                        )
                        oh_hi_src = oh_lo_src
                        oh_lo_base, oh_hi_base = 0, nlo * P
                    else:
                        oh_lo_src = ohp.tile([P, gw * P], b16, tag="onehot")
                        nc.vector.tensor_tensor(
                            out=oh_lo_src[:, 0:nsub * P].rearrange(
                                "p (k s) -> p k s", k=nsub),
                            in0=seg_t[:, g * gw:g * gw + nsub].to_broadcast(
                                [P, nsub, P]),
                            in1=iota_t[:, 0:nsub * P].rearrange(
                                "p (k s) -> p k s", k=nsub),
                            op=mybir.AluOpType.is_equal,
                        )
                        oh_hi_src = oh_lo_src
                        oh_lo_base, oh_hi_base = 0, nlo * P

                    # ---- own features for the group (contiguous rows) ----
                    x_g = xgp.tile([P, GS, H], b16, tag="xg")
                    nc.sync.dma_start(
                        out=x_g[:, 0:ng, :],
                        in_=own[g * GS * P:(g * GS + ng) * P, :]
                        .rearrange("(k p) h -> p k h", p=P))

                    # ---- output staging tile for the group ----
                    if not last:
                        h_g = hgp.tile([P, GS, H], b16, tag="hg")

                    use_dr = DR_ and tabdt == fp8d and ohdt == fp8d
                    for i, b in enumerate(blocks):
                        # ---- segmented sum (lo subtiles, then hi) ----
                        psum_agg = pa.tile([P, H], f32, tag="pagg")
                        runs = ((oh_lo_src, oh_lo_base, m_lo,
                                 i * sub_lo, sub_lo),
                                (oh_hi_src, oh_hi_base, m_hi,
                                 i * sub_hi, sub_hi))
                        if use_dr:
                            nmm = sum(cnt // 2 + cnt % 2 for *_, cnt in runs)
                        else:
                            nmm = sub_lo + sub_hi
                        jj = 0
                        for oh_s, oh_b, mt, j0, cnt in runs:
                            k = 0
                            while k < cnt:
                                j = j0 + k
                                c0 = oh_b + j * P
                                if use_dr and k + 1 < cnt:
                                    nc.tensor.matmul(
                                        out=psum_agg[:],
                                        lhsT=oh_s[:, c0:c0 + 2 * P].rearrange(
                                            "p (t s) -> p t s", t=2),
                                        rhs=mt[:, j:j + 2, :],
                                        start=(jj == 0),
                                        stop=(jj == nmm - 1),
                                        perf_mode=mybir.MatmulPerfMode
                                        .DoubleRow,
                                    )
                                    k += 2
                                else:
                                    nc.tensor.matmul(
                                        out=psum_agg[:],
                                        lhsT=oh_s[:, c0:c0 + P],
                                        rhs=mt[:, j, :],
                                        start=(jj == 0),
                                        stop=(jj == nmm - 1),
                                    )
                                    k += 1
                                jj += 1

                        # ---- mean (1/deg) on Act engine ----
                        agg_bf = actp.tile([P, H], b16, tag="aggbf")
                        nc.scalar.activation(
                            agg_bf[:], psum_agg[:], AT.Copy,
                            scale=recip_t[:, b:b + 1],
                        )

                        # ---- transposes (agg | x), feature-major chunks ----
                        tr_ps = ptr.tile([P, 4, P], b16, tag="trps")
                        nc.tensor.transpose(out=tr_ps[:, 0, :],
                                            in_=agg_bf[:, 0:P],
                                            identity=ident_t[:])
                        nc.tensor.transpose(out=tr_ps[:, 1, :],
                                            in_=agg_bf[:, P:H],
                                            identity=ident_t[:])
                        nc.tensor.transpose(out=tr_ps[:, 2, :],
                                            in_=x_g[:, i, 0:P],
                                            identity=ident_t[:])
                        nc.tensor.transpose(out=tr_ps[:, 3, :],
                                            in_=x_g[:, i, P:H],
                                            identity=ident_t[:])
                        actT = actp.tile([P, 4, P], b16, tag="actT")
                        nc.vector.tensor_copy(out=actT[:], in_=tr_ps[:])

                        # ---- dense: zz = agg@Wl.T + x@Wr.T + b' ----
                        psum_out = po.tile([P, H], f32, tag="pout")
                        wb = layer * 4
                        for i4 in range(4):
                            nc.tensor.matmul(
                                out=psum_out[:],
                                lhsT=actT[:, i4, :],
                                rhs=wts_t[:, wb + i4, :],
                                start=(i4 == 0),
                                stop=False,
                            )
                        nc.tensor.matmul(
                            out=psum_out[:],
                            lhsT=ones_t[:],
                            rhs=bias_t[:, layer, :],
                            start=False,
                            stop=True,
                        )

                        # ---- shifted ELU: h' = max(zz, exp(min(zz,1)-1)) ----
                        if ACT_ELU_:
                            zz_t = elup.tile([P, H], f32 if last else b16,
                                             tag="zz")
                            nc.scalar.activation(zz_t[:], psum_out[:], AT.Copy)
                            m_t = elup.tile([P, H], b16, tag="m")
                            nc.vector.tensor_scalar(
                                out=m_t[:], in0=zz_t[:],
                                scalar1=1.0, scalar2=-1.0,
                                op0=mybir.AluOpType.min,
                                op1=mybir.AluOpType.add,
                            )
                            e_t = elup.tile([P, H], b16, tag="e")
                            nc.scalar.activation(e_t[:], m_t[:], AT.Exp)
                            if last:
                                ot = elup.tile([P, H], f32, tag="ot")
                                nc.vector.tensor_tensor(
                                    out=ot[:], in0=zz_t[:], in1=e_t[:],
                                    op=mybir.AluOpType.max,
                                )
                                oo = elup.tile([P, H], f32, tag="oo")
                                nc.vector.tensor_scalar(
                                    out=oo[:], in0=ot[:],
                                    scalar1=-1.0, scalar2=None,
                                    op0=mybir.AluOpType.add,
                                )
                                nc.sync.dma_start(
                                    out=out_d[b * P:(b + 1) * P, :],
                                    in_=oo[:])
                            else:
                                nc.vector.tensor_tensor(
                                    out=h_g[:, i, :], in0=zz_t[:], in1=e_t[:],
                                    op=mybir.AluOpType.max,
                                )
                        else:
                            m_t = elup.tile([P, H], b16, tag="m")
                            nc.vector.tensor_scalar(
                                out=m_t[:], in0=psum_out[:],
                                scalar1=1.0, scalar2=-1.0,
                                op0=mybir.AluOpType.min,
                                op1=mybir.AluOpType.add,
                            )
                            e_t = elup.tile([P, H], f32, tag="e")
                            nc.scalar.activation(e_t[:], m_t[:], AT.Exp)
                            if last:
                                ot = elup.tile([P, H], f32, tag="ot")
                                nc.vector.tensor_tensor(
                                    out=ot[:], in0=psum_out[:], in1=e_t[:],
                                    op=mybir.AluOpType.max,
                                )
                                oo = elup.tile([P, H], f32, tag="oo")
                                nc.vector.tensor_scalar(
                                    out=oo[:], in0=ot[:],
                                    scalar1=-1.0, scalar2=None,
                                    op0=mybir.AluOpType.add,
                                )
                                nc.sync.dma_start(
                                    out=out_d[b * P:(b + 1) * P, :],
                                    in_=oo[:])
                            else:
                                nc.vector.tensor_tensor(
                                    out=h_g[:, i, :], in0=psum_out[:],
                                    in1=e_t[:],
                                    op=mybir.AluOpType.max,
                                )

                    # ---- group-batched writes ----
                    rows = slice(g * GS * P, (g * GS + ng) * P)
                    if not last:
                        nc.sync.dma_start(
                            out=h_own[layer][rows, :].rearrange(
                                "(k p) h -> p k h", p=P),
                            in_=h_g[:, 0:ng, :])
                        if TAB8_:
                            # centered table: store h = h' - 1 so fp8 error
                            # scales with |h| (small) instead of |h + 1|
                            h8_g = hgp.tile([P, GS, H], fp8d, tag="h8g")
                            nc.scalar.activation(
                                h8_g[:, 0:ng, :].rearrange("p k h -> p (k h)"),
                                h_g[:, 0:ng, :].rearrange("p k h -> p (k h)"),
                                AT.Copy, bias=-1.0,
                            )
                            nc.sync.dma_start(
                                out=h_tab[layer][rows, :].rearrange(
                                    "(k p) h -> p k h", p=P),
                                in_=h8_g[:, 0:ng, :])

                if layer < min(2, N_LAYERS_ - 1) and USE_CC_:
                    nc.gpsimd.collective_compute(
                        "AllGather",
                        mybir.AluOpType.bypass,
                        ins=[h_tab[layer][:]],
                        outs=[h_full[layer][:]],
                        replica_groups=[list(range(NC))],
                    )

    nc.compile()
    return nc


_CACHE = {}


def _get_program(sub_lo: int, sub_hi: int, shift: bool):
    key = (sub_lo, sub_hi, shift)
    if key not in _CACHE:
        _CACHE[key] = _build(sub_lo, sub_hi, shift)
    return _CACHE[key]


def _make_in_maps(inputs: dict, pp: dict) -> list:
    x = np.asarray(inputs["x"], dtype=np.float32)
    st = pp["st"]
    gw = GS * st
    table_row = pp["table_row"]
    shift = pp["shift_ok"]

    # permuted, padded table (fp8 or bf16) + bf16 own-feature shards
    xtab = np.zeros((TAB, H), dtype=f8 if XTAB8 else bf16)
    xtab[table_row] = x.astype(xtab.dtype)
    xown = np.zeros((TAB, H), dtype=bf16)
    xown[table_row] = x.astype(bf16)

    # weights: per layer [WlT chunk0, WlT chunk1, WrT chunk0, WrT chunk1]
    wchunks = []
    for l in range(3):
        for name in (f"Wl{l + 1}", f"Wr{l + 1}"):
            WT = np.asarray(inputs[name], dtype=np.float32).T.astype(bf16)
            wchunks.append(WT[0:P, :])
            wchunks.append(WT[P:H, :])
    wts = np.concatenate(wchunks, axis=0)  # [12*128, 256]

    # bias with the ELU/shift folds (device computes zz = z + 1):
    #   layer 0: b + 1
    #   layer 1,2 with centered fp8 h-table (stores h):  b - Wr.sum(1) + 1
    #     (only the x-path input h' = h + 1 needs correcting)
    #   layer 1,2 with bf16 table (stores h' = h + 1): b - Wl.sum - Wr.sum + 1
    biases = []
    for l in range(3):
        b = np.asarray(inputs[f"bl{l + 1}"], dtype=np.float32).copy()
        if l > 0:
            b -= np.asarray(inputs[f"Wr{l + 1}"], dtype=np.float32).sum(axis=1)
            if not TAB8:
                assert shift, "unshifted bf16 table path removed"
                b -= np.asarray(inputs[f"Wl{l + 1}"],
                                dtype=np.float32).sum(axis=1)
        b += 1.0
        biases.append(b)
    bias = np.stack(biases).astype(bf16)

    in_maps = []
    for c in range(NC):
        m = {
            "xtab": xtab,
            "xown": xown[c * STRIDE:(c + 1) * STRIDE],
            "wts": wts,
            "bias": bias,
            "idxall": pp["idx_all"][c],
            "recipall": pp["recip_all"][c],
        }
        if HOST_OH:
            m["ohloall"] = pp["oh_lo_all"][c]
            m["ohhiall"] = pp["oh_hi_all"][c]
        else:
            m["segall"] = pp["seg_all"][c]
            m["iotarep"] = np.tile(
                np.arange(P, dtype=np.float32), (P, gw)).astype(bf16)
        in_maps.append(m)
    return in_maps


def run(inputs: dict, trace: bool = False):
    """Returns (output [N_NODES, H] float32, exec_time_ns or None)."""
    edge_index = np.asarray(inputs["edge_index"])
    pp = _preprocess(edge_index)
    table_row = pp["table_row"]
    in_maps = _make_in_maps(inputs, pp)
    nc = _get_program(pp["sub_lo"], pp["sub_hi"], pp["shift_ok"])

    res = run_bass_kernel_spmd(nc, in_maps, core_ids=list(range(NC)),
                               trace=trace)

    out_full = np.empty((N_NODES, H), dtype=np.float32)
    for c in range(NC):
        shard = res.results[c]["out"]  # [STRIDE, H]
        rows = table_row - c * STRIDE
        mask = (rows >= 0) & (rows < STRIDE)
        out_full[mask] = shard[rows[mask]]
    return out_full, res.exec_time_ns


def kernel(**inputs) -> np.ndarray:
    out, _ = run(inputs)
    return out


# revision 42
# speedup vs baseline: 1.2338x; 1.2338x over previous
"""3-layer GraphSAGE (mean aggregation) on 8 TRN2 NeuronCores.

Self-contained: hardcoded problem shapes (N=50000, E=800000, H=256, 3 layers).

Strategy
--------
Host side (numpy, inside kernel()):
  * degree-balanced assignment of nodes to 8 cores x 49 blocks of <=128 nodes
  * permuted "table" layout: table row = core*6272 + block*128 + pos
  * blocks processed in GROUPS of GS=7 (uniform 7 groups); per-group edge
    lists as int16 gather indices (lo/hi split at row 32768 for int16 range),
    padded to uniform subtile counts so all 8 cores run one SPMD program
  * the segment-sum one-hot matrix (edge -> within-block position) is
    layer-invariant, so it is built on the HOST in fp8 (0/1 exact) and kept
    RESIDENT in SBUF for all three layers
  * tables gathered from are fp8; inter-layer tables store CENTERED h
    (= h' - 1 = raw ELU output, small magnitudes) so fp8 quantization error
    shrinks ~2x; the +1 shifts / Wr-sum corrections fold into the biases
  * ELU computed as max(z+1, exp(min(z+1,1)-1)) - 1

Device side (Bass/Tile), per layer:
  gather fp8 messages (8-subtile calls; bigger SWDGE rings crash HW) ->
  segmented sum via fp8 DoubleRow TensorE matmuls (2 subtiles/instruction)
  against the resident one-hot -> 1/deg scale on Activation engine ->
  PE transposes -> dense bf16 matmuls against W^T (+ K=1 bias matmul) ->
  shifted ELU (Act copy/exp + DVE min/max on bf16) -> h' stays resident in
  SBUF (in-place per block); per group a centered fp8 copy is written to
  DRAM for the AllGather (Shared output) between layers.
"""

import os
import sys

sys.path.insert(0, "/opt/trn_rl_repo")

import numpy as np
import ml_dtypes

from concourse import bacc, bass, mybir, tile
from concourse.bass_utils import run_bass_kernel_spmd
from concourse.masks import make_identity

bf16 = ml_dtypes.bfloat16
f8 = ml_dtypes.float8_e4m3

N_NODES = 50000
N_EDGES = 800000
H = 256
NC = 8
P = 128
NB = 49                      # blocks per core
HI_BASE = 32768              # int16 index split point

# knobs (env-overridable for experiments; defaults are the shipping config)
GS = int(os.environ.get("GCN_GS", "7"))          # blocks per gather group
TAB8 = os.environ.get("GCN_TAB8", "1") == "1"    # fp8 h-tables + messages
XTAB8 = os.environ.get("GCN_XTAB8",
                       os.environ.get("GCN_TAB8", "1")) == "1"  # fp8 x table
OH8 = os.environ.get("GCN_OH8", "1") == "1"      # fp8 one-hot (0/1 exact)
HOST_OH = os.environ.get("GCN_HOST_OH", "1") == "1"  # host-built one-hot
OH_RES = os.environ.get("GCN_OH_RES", "auto")    # SBUF-resident lo one-hot:
# "auto" = only when messages are fp8 (bf16 messages + resident oh overflow)
ACT_ELU = os.environ.get("GCN_ACT_ELU", "1") == "1"  # ELU via Act engine
DR = os.environ.get("GCN_DR", "1") == "1"        # DoubleRow on fp8 layers
CC_SHARED = os.environ.get("GCN_CC_SHARED", "1") == "1"
GMAX = int(os.environ.get("GCN_GMAX", "8"))      # subtiles per gather call
DMA_SCRATCH = int(os.environ.get("GCN_SCRATCH", "16384"))  # SWDGE ring bytes
# NOTE: dynamic_dma_scratch_size is charged PER PARTITION in SBUF, and rings
# bigger than the default 16384 (1024-desc) / gather calls above 1024 indices
# crash real HW (NRT_EXEC_UNIT_UNRECOVERABLE) — keep 16384/GMAX=8.
N_LAYERS = int(os.environ.get("GCN_LAYERS", "3"))
USE_CC = os.environ.get("GCN_CC", "1") == "1"

NG = (NB + GS - 1) // GS     # groups per core
STRIDE = NB * P              # 6272 table rows per core
TAB = NC * STRIDE            # 50176 table rows


def _group_blocks(g: int) -> list:
    return list(range(g * GS, min((g + 1) * GS, NB)))


def _assign_blocks(deg: np.ndarray) -> np.ndarray:
    """Serpentine deal of nodes (sorted by degree desc) into NC*NB blocks."""
    nb_total = NC * NB
    order = np.argsort(-deg, kind="stable")
    block_of_node = np.empty(N_NODES, dtype=np.int64)
    pos = 0
    rnd = 0
    while pos < N_NODES:
        take = min(nb_total, N_NODES - pos)
        blocks = np.arange(nb_total) if rnd % 2 == 0 else np.arange(nb_total)[::-1]
        block_of_node[order[pos:pos + take]] = blocks[:take]
        pos += take
        rnd += 1
    return block_of_node


def _preprocess(edge_index: np.ndarray):
    """Graph preprocessing. Returns dict of host-side structures."""
    src = np.asarray(edge_index[0], dtype=np.int64)
    dst = np.asarray(edge_index[1], dtype=np.int64)
    deg = np.bincount(dst, minlength=N_NODES).astype(np.int64)
    shift_ok = bool(deg.min() >= 1)

    block_of_node = _assign_blocks(deg)

    # position of each node within its block; table row of each node
    order = np.lexsort((np.arange(N_NODES), block_of_node))
    pos_in_block = np.empty(N_NODES, dtype=np.int64)
    counts = np.zeros(NC * NB, dtype=np.int64)
    for n in order:
        b = block_of_node[n]
        pos_in_block[n] = counts[b]
        counts[b] += 1
    assert counts.max() <= P, f"block overflow: {counts.max()}"
    table_row = block_of_node * P + pos_in_block

    # edges grouped by destination block
    e_block = block_of_node[dst]
    e_seg = pos_in_block[dst]
    e_srcrow = table_row[src]

    sort_idx = np.argsort(e_block, kind="stable")
    e_block_s = e_block[sort_idx]
    e_seg_s = e_seg[sort_idx]
    e_srcrow_s = e_srcrow[sort_idx]
    blk_starts = np.searchsorted(e_block_s, np.arange(NC * NB + 1))

    lo_counts = np.empty(NC * NB, dtype=np.int64)
    hi_counts = np.empty(NC * NB, dtype=np.int64)
    for b in range(NC * NB):
        rows = e_srcrow_s[blk_starts[b]:blk_starts[b + 1]]
        lo_counts[b] = int((rows < HI_BASE).sum())
        hi_counts[b] = rows.shape[0] - lo_counts[b]
    sub_lo = int(np.ceil(lo_counts.max() / P))
    sub_hi = int(np.ceil(hi_counts.max() / P))
    st = sub_lo + sub_hi

    # per-core packed arrays, group layout:
    #   subtile order per group: [lo(b0)..lo(bN)][hi(b0)..hi(bN)]
    gw = GS * st                                  # subtiles per (full) group
    idx_all = np.zeros((NC, P, NG * gw * 8), dtype=np.int16)
    seg_all = np.full((NC, P, NG * gw), 200.0, dtype=np.float32)
    recip_all = np.zeros((NC, P, NB), dtype=np.float32)

    recip = (1.0 / np.maximum(deg, 1)).astype(np.float32)

    def pack16(flat: np.ndarray) -> np.ndarray:
        # dma_gather layout: unwrapped[k] = tile16[k % 16, k // 16]
        n = flat.shape[0]
        t = flat.reshape(n // 16, 16).T
        return np.tile(t, (8, 1))  # [128, n/16]

    def padded(rows, segs, nsub):
        r = np.zeros(nsub * P, dtype=np.int16)
        r[:rows.shape[0]] = rows.astype(np.int16)
        s = np.full(nsub * P, 200.0, dtype=np.float32)
        s[:segs.shape[0]] = segs.astype(np.float32)
        return r, s.reshape(nsub, P).T  # seg -> [P, nsub]

    for c in range(NC):
        for g in range(NG):
            blocks = _group_blocks(g)
            los, his = [], []
            for lb in blocks:
                b = c * NB + lb
                rows = e_srcrow_s[blk_starts[b]:blk_starts[b + 1]]
                segs = e_seg_s[blk_starts[b]:blk_starts[b + 1]]
                is_lo = rows < HI_BASE
                los.append(padded(rows[is_lo], segs[is_lo], sub_lo))
                his.append(padded(rows[~is_lo] - HI_BASE, segs[~is_lo], sub_hi))

            ng = len(blocks)
            ibase = g * gw * 8
            sbase = g * gw
            lo_flat = np.concatenate([r for r, _ in los])
            hi_flat = np.concatenate([r for r, _ in his])
            idx_all[c, :, ibase:ibase + ng * sub_lo * 8] = pack16(lo_flat)
            idx_all[c, :, ibase + ng * sub_lo * 8:
                    ibase + ng * st * 8] = pack16(hi_flat)
            seg_all[c, :, sbase:sbase + ng * sub_lo] = np.concatenate(
                [s for _, s in los], axis=1)
            seg_all[c, :, sbase + ng * sub_lo:sbase + ng * st] = np.concatenate(
                [s for _, s in his], axis=1)

            for lb in blocks:
                b = c * NB + lb
                nodes_here = np.where(block_of_node == b)[0]
                recip_all[c, pos_in_block[nodes_here], lb] = recip[nodes_here]

    out = dict(
        table_row=table_row, sub_lo=sub_lo, sub_hi=sub_hi, st=st,
        idx_all=idx_all, seg_all=seg_all.astype(bf16), recip_all=recip_all,
        shift_ok=shift_ok,
    )

    if HOST_OH:
        # host-built one-hot: oh[c, p, j, s] = (seg(edge p of subtile j) == s)
        # in fp8 (0/1 exact), split into lo/hi subtile parts per group
        oh = (seg_all[..., None] == np.arange(P, dtype=np.float32)) \
            .astype(f8 if OH8 else bf16)
        oh = oh.reshape(NC, P, NG, gw, P)
        SL, SH = GS * sub_lo, GS * sub_hi
        oh_lo = np.zeros((NC, P, NG, SL, P), dtype=oh.dtype)
        oh_hi = np.zeros((NC, P, NG, SH, P), dtype=oh.dtype)
        for g in range(NG):
            ng = len(_group_blocks(g))
            nlo, nhi = ng * sub_lo, ng * sub_hi
            oh_lo[:, :, g, :nlo] = oh[:, :, g, :nlo]
            oh_hi[:, :, g, :nhi] = oh[:, :, g, nlo:nlo + nhi]
        out["oh_lo_all"] = oh_lo.reshape(NC, P, NG * SL * P)
        out["oh_hi_all"] = oh_hi.reshape(NC, P, NG * SH * P)
    return out


def _group_subtiles(i: int, ng: int, sub_lo: int, sub_hi: int) -> list:
    """Subtile columns of block i (0-based within group) in a group of ng."""
    lo = list(range(i * sub_lo, (i + 1) * sub_lo))
    hi = [ng * sub_lo + i * sub_hi + j for j in range(sub_hi)]
    return lo + hi


def _build(sub_lo: int, sub_hi: int, shift: bool,
           use_cc: bool | None = None,
           cc_shared: bool | None = None, n_layers: int | None = None,
           gmax: int | None = None, scratch: int | None = None,
           tab8: bool | None = None, xtab8: bool | None = None,
           oh8: bool | None = None, dr: bool | None = None,
           host_oh: bool | None = None,
           oh_res: bool | None = None, act_elu: bool | None = None):
    """Build the SPMD Bass program. Returns compiled nc."""
    USE_CC_ = USE_CC if use_cc is None else use_cc
    CC_SHARED_ = CC_SHARED if cc_shared is None else cc_shared
    N_LAYERS_ = N_LAYERS if n_layers is None else n_layers
    GMAX_ = GMAX if gmax is None else gmax
    SCRATCH_ = DMA_SCRATCH if scratch is None else scratch
    TAB8_ = TAB8 if tab8 is None else tab8
    XTAB8_ = XTAB8 if xtab8 is None else xtab8
    OH8_ = OH8 if oh8 is None else oh8
    DR_ = (DR if dr is None else dr) and OH8_
    HOST_OH_ = HOST_OH if host_oh is None else host_oh
    if oh_res is None:
        oh_res = (TAB8_ and XTAB8_) if OH_RES == "auto" else OH_RES == "1"
    OH_RES_ = oh_res and HOST_OH_
    ACT_ELU_ = ACT_ELU if act_elu is None else act_elu

    st = sub_lo + sub_hi
    gw = GS * st
    SL = GS * sub_lo            # lo subtiles per full group
    SH = GS * sub_hi            # hi subtiles per full group
    f32 = mybir.dt.float32
    b16 = mybir.dt.bfloat16
    fp8d = mybir.dt.float8e4
    xdt = fp8d if XTAB8_ else b16    # layer-0 table (x) dtype
    hdt = fp8d if TAB8_ else b16     # inter-layer table (h) dtype
    ohdt = fp8d if OH8_ else b16
    assert xdt == hdt, "per-layer message dtypes not supported yet"
    tabdt = xdt
    AT = mybir.ActivationFunctionType

    nc = bacc.Bacc("TRN2", target_bir_lowering=False, debug=False,
                   enable_asserts=True, num_devices=NC,
                   dynamic_dma_scratch_size=SCRATCH_)

    xtab_d = nc.dram_tensor("xtab", [TAB, H], tabdt, kind="ExternalInput")
    xown_d = nc.dram_tensor("xown", [STRIDE, H], b16, kind="ExternalInput")
    wts_d = nc.dram_tensor("wts", [12 * P, H], b16, kind="ExternalInput")
    bias_d = nc.dram_tensor("bias", [3, H], b16, kind="ExternalInput")
    idx_d = nc.dram_tensor("idxall", [P, NG * gw * 8], mybir.dt.int16,
                           kind="ExternalInput")
    recip_d = nc.dram_tensor("recipall", [P, NB], f32, kind="ExternalInput")
    if HOST_OH_:
        ohlo_d = nc.dram_tensor("ohloall", [P, NG * SL * P], ohdt,
                                kind="ExternalInput")
        ohhi_d = nc.dram_tensor("ohhiall", [P, NG * SH * P], ohdt,
                                kind="ExternalInput")
    else:
        seg_d = nc.dram_tensor("segall", [P, NG * gw], b16,
                               kind="ExternalInput")
        iota_d = nc.dram_tensor("iotarep", [P, gw * P], b16,
                                kind="ExternalInput")

    out_d = nc.dram_tensor("out", [STRIDE, H], f32, kind="ExternalOutput")

    with tile.TileContext(nc) as tc:
        with (
            tc.tile_pool(name="const", bufs=1) as cp,
            tc.tile_pool(name="mlo", bufs=2) as mlo,
            tc.tile_pool(name="mhi", bufs=1 if OH_RES_ else 2) as mhi,
            tc.tile_pool(name="ohp", bufs=2) as ohp,
            tc.tile_pool(name="idxp", bufs=2) as idxp,
            tc.tile_pool(name="hgp", bufs=2) as hgp,
            tc.tile_pool(name="actp", bufs=2) as actp,
            tc.tile_pool(name="elup", bufs=2) as elup,
            tc.tile_pool(name="pa", bufs=3, space="PSUM") as pa,
            tc.tile_pool(name="po", bufs=2, space="PSUM") as po,
            tc.tile_pool(name="ptr", bufs=2, space="PSUM") as ptr,
            tc.tile_pool(name="dram", bufs=1, space="DRAM") as dr,
        ):
            # ---- resident constants ----
            recip_t = cp.tile([P, NB], f32)
            nc.sync.dma_start(out=recip_t[:], in_=recip_d[:])
            if HOST_OH_ and OH_RES_:
                # SBUF-resident one-hot (layer-invariant), loaded once.
                ohlo_t = cp.tile([P, NG * SL * P], ohdt)
                ohhi_t = cp.tile([P, NG * SH * P], ohdt)
                for g in range(NG):
                    nc.scalar.dma_start(
                        out=ohlo_t[:, g * SL * P:(g + 1) * SL * P],
                        in_=ohlo_d[:, g * SL * P:(g + 1) * SL * P])
                    nc.scalar.dma_start(
                        out=ohhi_t[:, g * SH * P:(g + 1) * SH * P],
                        in_=ohhi_d[:, g * SH * P:(g + 1) * SH * P])
            if not HOST_OH_:
                seg_t = cp.tile([P, NG * gw], b16)
                nc.sync.dma_start(out=seg_t[:], in_=seg_d[:])
                iota_t = cp.tile([P, gw * P], b16)
                nc.sync.dma_start(out=iota_t[:], in_=iota_d[:])
            wts_t = cp.tile([P, 12, H], b16)
            nc.sync.dma_start(
                out=wts_t[:], in_=wts_d[:].rearrange("(c k) h -> k c h", k=P)
            )
            bias_t = cp.tile([1, 3, H], b16)
            nc.sync.dma_start(
                out=bias_t[:], in_=bias_d[:].rearrange("(a c) h -> a c h", a=1)
            )
            ones_t = cp.tile([1, P], b16)
            nc.vector.memset(ones_t[:], 1.0)
            ident_t = cp.tile([P, P], b16)
            make_identity(nc, ident_t[:])

            # own-node activations stay resident in SBUF across layers
            # (h' = h + 1, bf16), updated in place block by block
            h_sb = cp.tile([P, NB, H], b16)
            nc.sync.dma_start(
                out=h_sb[:], in_=xown_d[:].rearrange("(k p) h -> p k h", p=P))

            # internal DRAM for inter-layer activations (AllGather path)
            addr_space = "Shared" if CC_SHARED_ else "Local"
            h_tab = [
                dr.tile([STRIDE, H], tabdt, tag=f"h_tab{i}", name=f"h_tab{i}")
                for i in range(2)
            ]
            h_full = [
                dr.tile([TAB, H], tabdt, tag=f"h_full{i}", name=f"h_full{i}",
                        addr_space=addr_space)
                for i in range(2)
            ]

            for layer in range(N_LAYERS_):
                last = layer == N_LAYERS_ - 1
                if layer == 0:
                    src_tab = xtab_d
                else:
                    src_tab = h_full[layer - 1] if USE_CC_ else xtab_d
                tab_lo = src_tab[:HI_BASE, :]
                tab_hi = src_tab[HI_BASE:, :]

                for g in range(NG):
                    blocks = _group_blocks(g)
                    ng = len(blocks)
                    nsub = ng * st
                    nlo = ng * sub_lo
                    nhi = ng * sub_hi

                    # ---- per-group gather indices ----
                    idx_t = idxp.tile([P, gw * 8], mybir.dt.int16, tag="idx")
                    nc.scalar.dma_start(
                        out=idx_t[:, 0:nsub * 8],
                        in_=idx_d[:, g * gw * 8:(g * gw + nsub) * 8])

                    # ---- gather messages (lo/hi into separate tiles) ----
                    m_lo = mlo.tile([P, SL, H], tabdt, tag="mlo")
                    m_hi = mhi.tile([P, SH, H], tabdt, tag="mhi")
                    for tab, mt, s0, s1 in ((tab_lo, m_lo, 0, nlo),
                                            (tab_hi, m_hi, nlo, nsub)):
                        step = (s1 - s0) if GMAX_ <= 0 else GMAX_
                        for g0 in range(s0, s1, step):
                            g1 = min(g0 + step, s1)
                            nc.gpsimd.dma_gather(
                                mt[:, g0 - s0:g1 - s0, :], tab,
                                idx_t[:, g0 * 8:g1 * 8],
                                (g1 - g0) * P, (g1 - g0) * P, H,
                                single_packet=True,
                            )

                    # ---- one-hot: resident, streamed, or DVE-built ----
                    if HOST_OH_ and OH_RES_:
                        oh_lo_src, oh_lo_base = ohlo_t, g * SL * P
                        oh_hi_src, oh_hi_base = ohhi_t, g * SH * P
                    elif HOST_OH_:
                        oh_lo_src = ohp.tile([P, gw * P], ohdt, tag="onehot")
                        nc.scalar.dma_start(
                            out=oh_lo_src[:, 0:nlo * P],
                            in_=ohlo_d[:, g * SL * P:(g * SL + nlo) * P],
                        )
                        nc.scalar.dma_start(
                            out=oh_lo_src[:, nlo * P:nsub * P],
                            in_=ohhi_d[:, g * SH * P:(g * SH + nhi) * P],
                        )
                        oh_hi_src = oh_lo_src
                        oh_lo_base, oh_hi_base = 0, nlo * P
                    else:
                        oh_lo_src = ohp.tile([P, gw * P], b16, tag="onehot")
                        nc.vector.tensor_tensor(
                            out=oh_lo_src[:, 0:nsub * P].rearrange(
                                "p (k s) -> p k s", k=nsub),
                            in0=seg_t[:, g * gw:g * gw + nsub].to_broadcast(
                                [P, nsub, P]),
                            in1=iota_t[:, 0:nsub * P].rearrange(
                                "p (k s) -> p k s", k=nsub),
                            op=mybir.AluOpType.is_equal,
                        )
                        oh_hi_src = oh_lo_src
                        oh_lo_base, oh_hi_base = 0, nlo * P

                    use_dr = DR_ and tabdt == fp8d and ohdt == fp8d
                    for i, b in enumerate(blocks):
                        # ---- segmented sum (lo subtiles, then hi) ----
                        psum_agg = pa.tile([P, H], f32, tag="pagg")
                        runs = ((oh_lo_src, oh_lo_base, m_lo,
                                 i * sub_lo, sub_lo),
                                (oh_hi_src, oh_hi_base, m_hi,
                                 i * sub_hi, sub_hi))
                        if use_dr:
                            nmm = sum(cnt // 2 + cnt % 2 for *_, cnt in runs)
                        else:
                            nmm = sub_lo + sub_hi
                        jj = 0
                        for oh_s, oh_b, mt, j0, cnt in runs:
                            k = 0
                            while k < cnt:
                                j = j0 + k
                                c0 = oh_b + j * P
                                if use_dr and k + 1 < cnt:
                                    nc.tensor.matmul(
                                        out=psum_agg[:],
                                        lhsT=oh_s[:, c0:c0 + 2 * P].rearrange(
                                            "p (t s) -> p t s", t=2),
                                        rhs=mt[:, j:j + 2, :],
                                        start=(jj == 0),
                                        stop=(jj == nmm - 1),
                                        perf_mode=mybir.MatmulPerfMode
                                        .DoubleRow,
                                    )
                                    k += 2
                                else:
                                    nc.tensor.matmul(
                                        out=psum_agg[:],
                                        lhsT=oh_s[:, c0:c0 + P],
                                        rhs=mt[:, j, :],
                                        start=(jj == 0),
                                        stop=(jj == nmm - 1),
                                    )
                                    k += 1
                                jj += 1

                        # ---- mean (1/deg) on Act engine ----
                        agg_bf = actp.tile([P, H], b16, tag="aggbf")
                        nc.scalar.activation(
                            agg_bf[:], psum_agg[:], AT.Copy,
                            scale=recip_t[:, b:b + 1],
                        )

                        # ---- transposes (agg | x), feature-major chunks ----
                        tr_ps = ptr.tile([P, 4, P], b16, tag="trps")
                        nc.tensor.transpose(out=tr_ps[:, 0, :],
                                            in_=agg_bf[:, 0:P],
                                            identity=ident_t[:])
                        nc.tensor.transpose(out=tr_ps[:, 1, :],
                                            in_=agg_bf[:, P:H],
                                            identity=ident_t[:])
                        nc.tensor.transpose(out=tr_ps[:, 2, :],
                                            in_=h_sb[:, b, 0:P],
                                            identity=ident_t[:])
                        nc.tensor.transpose(out=tr_ps[:, 3, :],
                                            in_=h_sb[:, b, P:H],
                                            identity=ident_t[:])
                        actT = actp.tile([P, 4, P], b16, tag="actT")
                        nc.vector.tensor_copy(out=actT[:], in_=tr_ps[:])

                        # ---- dense: zz = agg@Wl.T + x@Wr.T + b' ----
                        psum_out = po.tile([P, H], f32, tag="pout")
                        wb = layer * 4
                        for i4 in range(4):
                            nc.tensor.matmul(
                                out=psum_out[:],
                                lhsT=actT[:, i4, :],
                                rhs=wts_t[:, wb + i4, :],
                                start=(i4 == 0),
                                stop=False,
                            )
                        nc.tensor.matmul(
                            out=psum_out[:],
                            lhsT=ones_t[:],
                            rhs=bias_t[:, layer, :],
                            start=False,
                            stop=True,
                        )

                        # ---- shifted ELU: h' = max(zz, exp(min(zz,1)-1)) ----
                        if ACT_ELU_:
                            zz_t = elup.tile([P, H], f32 if last else b16,
                                             tag="zz")
                            nc.scalar.activation(zz_t[:], psum_out[:], AT.Copy)
                            m_t = elup.tile([P, H], b16, tag="m")
                            nc.vector.tensor_scalar(
                                out=m_t[:], in0=zz_t[:],
                                scalar1=1.0, scalar2=-1.0,
                                op0=mybir.AluOpType.min,
                                op1=mybir.AluOpType.add,
                            )
                            e_t = elup.tile([P, H], b16, tag="e")
                            nc.scalar.activation(e_t[:], m_t[:], AT.Exp)
                            if last:
                                ot = elup.tile([P, H], f32, tag="ot")
                                nc.vector.tensor_tensor(
                                    out=ot[:], in0=zz_t[:], in1=e_t[:],
                                    op=mybir.AluOpType.max,
                                )
                                oo = elup.tile([P, H], f32, tag="oo")
                                nc.vector.tensor_scalar(
                                    out=oo[:], in0=ot[:],
                                    scalar1=-1.0, scalar2=None,
                                    op0=mybir.AluOpType.add,
                                )
                                nc.sync.dma_start(
                                    out=out_d[b * P:(b + 1) * P, :],
                                    in_=oo[:])
                            else:
                                nc.vector.tensor_tensor(
                                    out=h_sb[:, b, :], in0=zz_t[:],
                                    in1=e_t[:],
                                    op=mybir.AluOpType.max,
                                )
                        else:
                            m_t = elup.tile([P, H], b16, tag="m")
                            nc.vector.tensor_scalar(
                                out=m_t[:], in0=psum_out[:],
                                scalar1=1.0, scalar2=-1.0,
                                op0=mybir.AluOpType.min,
                                op1=mybir.AluOpType.add,
                            )
                            e_t = elup.tile([P, H], f32, tag="e")
                            nc.scalar.activation(e_t[:], m_t[:], AT.Exp)
                            if last:
                                ot = elup.tile([P, H], f32, tag="ot")
                                nc.vector.tensor_tensor(
                                    out=ot[:], in0=psum_out[:], in1=e_t[:],
                                    op=mybir.AluOpType.max,
                                )
                                oo = elup.tile([P, H], f32, tag="oo")
                                nc.vector.tensor_scalar(
                                    out=oo[:], in0=ot[:],
                                    scalar1=-1.0, scalar2=None,
                                    op0=mybir.AluOpType.add,
                                )
                                nc.sync.dma_start(
                                    out=out_d[b * P:(b + 1) * P, :],
                                    in_=oo[:])
                            else:
                                nc.vector.tensor_tensor(
                                    out=h_sb[:, b, :], in0=psum_out[:],
                                    in1=e_t[:],
                                    op=mybir.AluOpType.max,
                                )

                    # ---- group-batched table write (for AllGather) ----
                    rows = slice(g * GS * P, (g * GS + ng) * P)
                    ksl = slice(g * GS, g * GS + ng)
                    if not last:
                        if TAB8_:
                            # centered table: store h = h' - 1 so fp8 error
                            # scales with |h| (small) instead of |h + 1|
                            h8_g = hgp.tile([P, GS, H], fp8d, tag="h8g")
                            nc.scalar.activation(
                                h8_g[:, 0:ng, :].rearrange("p k h -> p (k h)"),
                                h_sb[:, ksl, :].rearrange("p k h -> p (k h)"),
                                AT.Copy, bias=-1.0,
                            )
                            nc.sync.dma_start(
                                out=h_tab[layer][rows, :].rearrange(
                                    "(k p) h -> p k h", p=P),
                                in_=h8_g[:, 0:ng, :])
                        else:
                            nc.sync.dma_start(
                                out=h_tab[layer][rows, :].rearrange(
                                    "(k p) h -> p k h", p=P),
                                in_=h_sb[:, ksl, :])

                if layer < min(2, N_LAYERS_ - 1) and USE_CC_:
                    nc.gpsimd.collective_compute(
                        "AllGather",
                        mybir.AluOpType.bypass,
                        ins=[h_tab[layer][:]],
                        outs=[h_full[layer][:]],
                        replica_groups=[list(range(NC))],
                    )

    nc.compile()
    return nc


_CACHE = {}


def _get_program(sub_lo: int, sub_hi: int, shift: bool):
    key = (sub_lo, sub_hi, shift)
    if key not in _CACHE:
        _CACHE[key] = _build(sub_lo, sub_hi, shift)
    return _CACHE[key]


def _make_in_maps(inputs: dict, pp: dict) -> list:
    x = np.asarray(inputs["x"], dtype=np.float32)
    st = pp["st"]
    gw = GS * st
    table_row = pp["table_row"]
    shift = pp["shift_ok"]

    # permuted, padded table (fp8 or bf16) + bf16 own-feature shards
    xtab = np.zeros((TAB, H), dtype=f8 if XTAB8 else bf16)
    xtab[table_row] = x.astype(xtab.dtype)
    xown = np.zeros((TAB, H), dtype=bf16)
    xown[table_row] = x.astype(bf16)

    # weights: per layer [WlT chunk0, WlT chunk1, WrT chunk0, WrT chunk1]
    wchunks = []
    for l in range(3):
        for name in (f"Wl{l + 1}", f"Wr{l + 1}"):
            WT = np.asarray(inputs[name], dtype=np.float32).T.astype(bf16)
            wchunks.append(WT[0:P, :])
            wchunks.append(WT[P:H, :])
    wts = np.concatenate(wchunks, axis=0)  # [12*128, 256]

    # bias with the ELU/shift folds (device computes zz = z + 1):
    #   layer 0: b + 1
    #   layer 1,2 with centered fp8 h-table (stores h):  b - Wr.sum(1) + 1
    #     (only the x-path input h' = h + 1 needs correcting)
    #   layer 1,2 with bf16 table (stores h' = h + 1): b - Wl.sum - Wr.sum + 1
    biases = []
    for l in range(3):
        b = np.asarray(inputs[f"bl{l + 1}"], dtype=np.float32).copy()
        if l > 0:
            b -= np.asarray(inputs[f"Wr{l + 1}"], dtype=np.float32).sum(axis=1)
            if not TAB8:
                assert shift, "unshifted bf16 table path removed"
                b -= np.asarray(inputs[f"Wl{l + 1}"],
                                dtype=np.float32).sum(axis=1)
        b += 1.0
        biases.append(b)
    bias = np.stack(biases).astype(bf16)

    in_maps = []
    for c in range(NC):
        m = {
            "xtab": xtab,
            "xown": xown[c * STRIDE:(c + 1) * STRIDE],
            "wts": wts,
            "bias": bias,
            "idxall": pp["idx_all"][c],
            "recipall": pp["recip_all"][c],
        }
        if HOST_OH:
            m["ohloall"] = pp["oh_lo_all"][c]
            m["ohhiall"] = pp["oh_hi_all"][c]
        else:
            m["segall"] = pp["seg_all"][c]
            m["iotarep"] = np.tile(
                np.arange(P, dtype=np.float32), (P, gw)).astype(bf16)
        in_maps.append(m)
    return in_maps


def run(inputs: dict, trace: bool = False):
    """Returns (output [N_NODES, H] float32, exec_time_ns or None)."""
    edge_index = np.asarray(inputs["edge_index"])
    pp = _preprocess(edge_index)
    table_row = pp["table_row"]
    in_maps = _make_in_maps(inputs, pp)
    nc = _get_program(pp["sub_lo"], pp["sub_hi"], pp["shift_ok"])

    res = run_bass_kernel_spmd(nc, in_maps, core_ids=list(range(NC)),
                               trace=trace)

    out_full = np.empty((N_NODES, H), dtype=np.float32)
    for c in range(NC):
        shard = res.results[c]["out"]  # [STRIDE, H]
        rows = table_row - c * STRIDE
        mask = (rows >= 0) & (rows < STRIDE)
        out_full[mask] = shard[rows[mask]]
    return out_full, res.exec_time_ns


def kernel(**inputs) -> np.ndarray:
    out, _ = run(inputs)
    return out


# revision 44
# speedup vs baseline: 1.5994x; 1.2963x over previous
"""3-layer GraphSAGE (mean aggregation) on 8 TRN2 NeuronCores.

Self-contained: hardcoded problem shapes (N=50000, E=800000, H=256, 3 layers).

Strategy
--------
Host side (numpy, inside kernel()):
  * degree-balanced assignment of nodes to 8 cores x 49 blocks of <=128 nodes
  * permuted "table" layout: table row = core*6272 + block*128 + pos
  * blocks processed in GROUPS of GS=7 (uniform 7 groups); per-group edge
    lists as int16 gather indices (lo/hi split at row 32768 for int16 range),
    padded to uniform subtile counts so all 8 cores run one SPMD program
  * the segment-sum one-hot matrix (edge -> within-block position) is
    layer-invariant, so it is built on the HOST in fp8 (0/1 exact) and kept
    RESIDENT in SBUF for all three layers
  * tables gathered from are fp8; inter-layer tables store CENTERED h
    (= h' - 1 = raw ELU output, small magnitudes) so fp8 quantization error
    shrinks ~2x; the +1 shifts / Wr-sum corrections fold into the biases
  * ELU computed as max(z+1, exp(min(z+1,1)-1)) - 1

Device side (Bass/Tile), per layer:
  gather fp8 messages (8-subtile calls; bigger SWDGE rings crash HW) ->
  segmented sum via fp8 DoubleRow TensorE matmuls (2 subtiles/instruction)
  against the resident one-hot -> 1/deg scale on Activation engine ->
  PE transposes -> dense bf16 matmuls against W^T (+ K=1 bias matmul) ->
  shifted ELU (Act copy/exp + DVE min/max on bf16) -> h' stays resident in
  SBUF (in-place per block); per group a centered fp8 copy is written to
  DRAM for the AllGather (Shared output) between layers.
"""

import os
import sys

sys.path.insert(0, "/opt/trn_rl_repo")

import numpy as np
import ml_dtypes

from concourse import bacc, bass, mybir, tile
from concourse.bass_utils import run_bass_kernel_spmd
from concourse.masks import make_identity

bf16 = ml_dtypes.bfloat16
f8 = ml_dtypes.float8_e4m3

N_NODES = 50000
N_EDGES = 800000
H = 256
NC = 8
P = 128
NB = 49                      # blocks per core
HI_BASE = 32768              # int16 index split point

# knobs (env-overridable for experiments; defaults are the shipping config)
GS = int(os.environ.get("GCN_GS", "7"))          # blocks per gather group
TAB8 = os.environ.get("GCN_TAB8", "1") == "1"    # fp8 h-tables + messages
XTAB8 = os.environ.get("GCN_XTAB8",
                       os.environ.get("GCN_TAB8", "1")) == "1"  # fp8 x table
OH8 = os.environ.get("GCN_OH8", "1") == "1"      # fp8 one-hot (0/1 exact)
HOST_OH = os.environ.get("GCN_HOST_OH", "1") == "1"  # host-built one-hot
OH_RES = os.environ.get("GCN_OH_RES", "auto")    # SBUF-resident lo one-hot:
# "auto" = only when messages are fp8 (bf16 messages + resident oh overflow)
ACT_ELU = os.environ.get("GCN_ACT_ELU", "1") == "1"  # ELU via Act engine
DR = os.environ.get("GCN_DR", "1") == "1"        # DoubleRow on fp8 layers
CC_SHARED = os.environ.get("GCN_CC_SHARED", "1") == "1"
GMAX = int(os.environ.get("GCN_GMAX", "8"))      # subtiles per gather call
DMA_SCRATCH = int(os.environ.get("GCN_SCRATCH", "16384"))  # SWDGE ring bytes
# NOTE: dynamic_dma_scratch_size is charged PER PARTITION in SBUF, and rings
# bigger than the default 16384 (1024-desc) / gather calls above 1024 indices
# crash real HW (NRT_EXEC_UNIT_UNRECOVERABLE) — keep 16384/GMAX=8.
N_LAYERS = int(os.environ.get("GCN_LAYERS", "3"))
USE_CC = os.environ.get("GCN_CC", "1") == "1"

NG = (NB + GS - 1) // GS     # groups per core
STRIDE = NB * P              # 6272 table rows per core
TAB = NC * STRIDE            # 50176 table rows


def _group_blocks(g: int) -> list:
    return list(range(g * GS, min((g + 1) * GS, NB)))


def _assign_blocks(deg: np.ndarray) -> np.ndarray:
    """Serpentine deal of nodes (sorted by degree desc) into NC*NB blocks."""
    nb_total = NC * NB
    order = np.argsort(-deg, kind="stable")
    block_of_node = np.empty(N_NODES, dtype=np.int64)
    pos = 0
    rnd = 0
    while pos < N_NODES:
        take = min(nb_total, N_NODES - pos)
        blocks = np.arange(nb_total) if rnd % 2 == 0 else np.arange(nb_total)[::-1]
        block_of_node[order[pos:pos + take]] = blocks[:take]
        pos += take
        rnd += 1
    return block_of_node


def _preprocess(edge_index: np.ndarray):
    """Graph preprocessing. Returns dict of host-side structures."""
    src = np.asarray(edge_index[0], dtype=np.int64)
    dst = np.asarray(edge_index[1], dtype=np.int64)
    deg = np.bincount(dst, minlength=N_NODES).astype(np.int64)
    shift_ok = bool(deg.min() >= 1)

    block_of_node = _assign_blocks(deg)

    # position of each node within its block; table row of each node
    order = np.lexsort((np.arange(N_NODES), block_of_node))
    pos_in_block = np.empty(N_NODES, dtype=np.int64)
    counts = np.zeros(NC * NB, dtype=np.int64)
    for n in order:
        b = block_of_node[n]
        pos_in_block[n] = counts[b]
        counts[b] += 1
    assert counts.max() <= P, f"block overflow: {counts.max()}"
    table_row = block_of_node * P + pos_in_block

    # edges grouped by destination block
    e_block = block_of_node[dst]
    e_seg = pos_in_block[dst]
    e_srcrow = table_row[src]

    sort_idx = np.argsort(e_block, kind="stable")
    e_block_s = e_block[sort_idx]
    e_seg_s = e_seg[sort_idx]
    e_srcrow_s = e_srcrow[sort_idx]
    blk_starts = np.searchsorted(e_block_s, np.arange(NC * NB + 1))

    lo_counts = np.empty(NC * NB, dtype=np.int64)
    hi_counts = np.empty(NC * NB, dtype=np.int64)
    for b in range(NC * NB):
        rows = e_srcrow_s[blk_starts[b]:blk_starts[b + 1]]
        lo_counts[b] = int((rows < HI_BASE).sum())
        hi_counts[b] = rows.shape[0] - lo_counts[b]
    sub_lo = int(np.ceil(lo_counts.max() / P))
    sub_hi = int(np.ceil(hi_counts.max() / P))
    st = sub_lo + sub_hi

    # per-core packed arrays, group layout:
    #   subtile order per group: [lo(b0)..lo(bN)][hi(b0)..hi(bN)]
    gw = GS * st                                  # subtiles per (full) group
    idx_all = np.zeros((NC, P, NG * gw * 8), dtype=np.int16)
    seg_all = np.full((NC, P, NG * gw), 200.0, dtype=np.float32)
    recip_all = np.zeros((NC, P, NB), dtype=np.float32)

    recip = (1.0 / np.maximum(deg, 1)).astype(np.float32)

    def pack16(flat: np.ndarray) -> np.ndarray:
        # dma_gather layout: unwrapped[k] = tile16[k % 16, k // 16]
        n = flat.shape[0]
        t = flat.reshape(n // 16, 16).T
        return np.tile(t, (8, 1))  # [128, n/16]

    def padded(rows, segs, nsub):
        r = np.zeros(nsub * P, dtype=np.int16)
        r[:rows.shape[0]] = rows.astype(np.int16)
        s = np.full(nsub * P, 200.0, dtype=np.float32)
        s[:segs.shape[0]] = segs.astype(np.float32)
        return r, s.reshape(nsub, P).T  # seg -> [P, nsub]

    for c in range(NC):
        for g in range(NG):
            blocks = _group_blocks(g)
            los, his = [], []
            for lb in blocks:
                b = c * NB + lb
                rows = e_srcrow_s[blk_starts[b]:blk_starts[b + 1]]
                segs = e_seg_s[blk_starts[b]:blk_starts[b + 1]]
                is_lo = rows < HI_BASE
                los.append(padded(rows[is_lo], segs[is_lo], sub_lo))
                his.append(padded(rows[~is_lo] - HI_BASE, segs[~is_lo], sub_hi))

            ng = len(blocks)
            ibase = g * gw * 8
            sbase = g * gw
            lo_flat = np.concatenate([r for r, _ in los])
            hi_flat = np.concatenate([r for r, _ in his])
            idx_all[c, :, ibase:ibase + ng * sub_lo * 8] = pack16(lo_flat)
            idx_all[c, :, ibase + ng * sub_lo * 8:
                    ibase + ng * st * 8] = pack16(hi_flat)
            seg_all[c, :, sbase:sbase + ng * sub_lo] = np.concatenate(
                [s for _, s in los], axis=1)
            seg_all[c, :, sbase + ng * sub_lo:sbase + ng * st] = np.concatenate(
                [s for _, s in his], axis=1)

            for lb in blocks:
                b = c * NB + lb
                nodes_here = np.where(block_of_node == b)[0]
                recip_all[c, pos_in_block[nodes_here], lb] = recip[nodes_here]

    out = dict(
        table_row=table_row, sub_lo=sub_lo, sub_hi=sub_hi, st=st,
        idx_all=idx_all, seg_all=seg_all.astype(bf16), recip_all=recip_all,
        shift_ok=shift_ok,
    )

    if HOST_OH:
        # host-built one-hot: oh[c, p, j, s] = (seg(edge p of subtile j) == s)
        # in fp8 (0/1 exact), split into lo/hi subtile parts per group
        oh = (seg_all[..., None] == np.arange(P, dtype=np.float32)) \
            .astype(f8 if OH8 else bf16)
        oh = oh.reshape(NC, P, NG, gw, P)
        SL, SH = GS * sub_lo, GS * sub_hi
        oh_lo = np.zeros((NC, P, NG, SL, P), dtype=oh.dtype)
        oh_hi = np.zeros((NC, P, NG, SH, P), dtype=oh.dtype)
        for g in range(NG):
            ng = len(_group_blocks(g))
            nlo, nhi = ng * sub_lo, ng * sub_hi
            oh_lo[:, :, g, :nlo] = oh[:, :, g, :nlo]
            oh_hi[:, :, g, :nhi] = oh[:, :, g, nlo:nlo + nhi]
        out["oh_lo_all"] = oh_lo.reshape(NC, P, NG * SL * P)
        out["oh_hi_all"] = oh_hi.reshape(NC, P, NG * SH * P)
    return out


def _group_subtiles(i: int, ng: int, sub_lo: int, sub_hi: int) -> list:
    """Subtile columns of block i (0-based within group) in a group of ng."""
    lo = list(range(i * sub_lo, (i + 1) * sub_lo))
    hi = [ng * sub_lo + i * sub_hi + j for j in range(sub_hi)]
    return lo + hi


def _build(sub_lo: int, sub_hi: int, shift: bool,
           use_cc: bool | None = None,
           cc_shared: bool | None = None, n_layers: int | None = None,
           gmax: int | None = None, scratch: int | None = None,
           tab8: bool | None = None, xtab8: bool | None = None,
           oh8: bool | None = None, dr: bool | None = None,
           host_oh: bool | None = None,
           oh_res: bool | None = None, act_elu: bool | None = None):
    """Build the SPMD Bass program. Returns compiled nc."""
    USE_CC_ = USE_CC if use_cc is None else use_cc
    CC_SHARED_ = CC_SHARED if cc_shared is None else cc_shared
    N_LAYERS_ = N_LAYERS if n_layers is None else n_layers
    GMAX_ = GMAX if gmax is None else gmax
    SCRATCH_ = DMA_SCRATCH if scratch is None else scratch
    TAB8_ = TAB8 if tab8 is None else tab8
    XTAB8_ = XTAB8 if xtab8 is None else xtab8
    OH8_ = OH8 if oh8 is None else oh8
    DR_ = (DR if dr is None else dr) and OH8_
    HOST_OH_ = HOST_OH if host_oh is None else host_oh
    if oh_res is None:
        oh_res = (TAB8_ and XTAB8_) if OH_RES == "auto" else OH_RES == "1"
    OH_RES_ = oh_res and HOST_OH_
    ACT_ELU_ = ACT_ELU if act_elu is None else act_elu

    st = sub_lo + sub_hi
    gw = GS * st
    SL = GS * sub_lo            # lo subtiles per full group
    SH = GS * sub_hi            # hi subtiles per full group
    f32 = mybir.dt.float32
    b16 = mybir.dt.bfloat16
    fp8d = mybir.dt.float8e4
    xdt = fp8d if XTAB8_ else b16    # layer-0 table (x) dtype
    hdt = fp8d if TAB8_ else b16     # inter-layer table (h) dtype
    ohdt = fp8d if OH8_ else b16
    assert xdt == hdt, "per-layer message dtypes not supported yet"
    tabdt = xdt
    AT = mybir.ActivationFunctionType

    nc = bacc.Bacc("TRN2", target_bir_lowering=False, debug=False,
                   enable_asserts=True, num_devices=NC,
                   dynamic_dma_scratch_size=SCRATCH_)

    xtab_d = nc.dram_tensor("xtab", [TAB, H], tabdt, kind="ExternalInput")
    xown_d = nc.dram_tensor("xown", [STRIDE, H], b16, kind="ExternalInput")
    wts_d = nc.dram_tensor("wts", [12 * P, H], b16, kind="ExternalInput")
    bias_d = nc.dram_tensor("bias", [3, H], b16, kind="ExternalInput")
    idx_d = nc.dram_tensor("idxall", [P, NG * gw * 8], mybir.dt.int16,
                           kind="ExternalInput")
    recip_d = nc.dram_tensor("recipall", [P, NB], f32, kind="ExternalInput")
    if HOST_OH_:
        ohlo_d = nc.dram_tensor("ohloall", [P, NG * SL * P], ohdt,
                                kind="ExternalInput")
        ohhi_d = nc.dram_tensor("ohhiall", [P, NG * SH * P], ohdt,
                                kind="ExternalInput")
    else:
        seg_d = nc.dram_tensor("segall", [P, NG * gw], b16,
                               kind="ExternalInput")
        iota_d = nc.dram_tensor("iotarep", [P, gw * P], b16,
                                kind="ExternalInput")

    out_d = nc.dram_tensor("out", [STRIDE, H], f32, kind="ExternalOutput")

    with tile.TileContext(nc) as tc:
        with (
            tc.tile_pool(name="const", bufs=1) as cp,
            tc.tile_pool(name="mlo", bufs=2) as mlo,
            tc.tile_pool(name="mhi", bufs=1 if OH_RES_ else 2) as mhi,
            tc.tile_pool(name="ohp", bufs=2) as ohp,
            tc.tile_pool(name="idxp", bufs=2) as idxp,
            tc.tile_pool(name="hgp", bufs=2) as hgp,
            tc.tile_pool(name="actp", bufs=3) as actp,
            tc.tile_pool(name="elup", bufs=2) as elup,
            tc.tile_pool(name="pa", bufs=4, space="PSUM") as pa,
            tc.tile_pool(name="po", bufs=2, space="PSUM") as po,
            tc.tile_pool(name="ptr", bufs=2, space="PSUM") as ptr,
            tc.tile_pool(name="dram", bufs=1, space="DRAM") as dr,
        ):
            # ---- resident constants ----
            recip_t = cp.tile([P, NB], f32)
            nc.sync.dma_start(out=recip_t[:], in_=recip_d[:])
            if HOST_OH_ and OH_RES_:
                # SBUF-resident one-hot (layer-invariant), loaded once.
                ohlo_t = cp.tile([P, NG * SL * P], ohdt)
                ohhi_t = cp.tile([P, NG * SH * P], ohdt)
                for g in range(NG):
                    nc.scalar.dma_start(
                        out=ohlo_t[:, g * SL * P:(g + 1) * SL * P],
                        in_=ohlo_d[:, g * SL * P:(g + 1) * SL * P])
                    nc.scalar.dma_start(
                        out=ohhi_t[:, g * SH * P:(g + 1) * SH * P],
                        in_=ohhi_d[:, g * SH * P:(g + 1) * SH * P])
            if not HOST_OH_:
                seg_t = cp.tile([P, NG * gw], b16)
                nc.sync.dma_start(out=seg_t[:], in_=seg_d[:])
                iota_t = cp.tile([P, gw * P], b16)
                nc.sync.dma_start(out=iota_t[:], in_=iota_d[:])
            wts_t = cp.tile([P, 12, H], b16)
            nc.sync.dma_start(
                out=wts_t[:], in_=wts_d[:].rearrange("(c k) h -> k c h", k=P)
            )
            bias_t = cp.tile([1, 3, H], b16)
            nc.sync.dma_start(
                out=bias_t[:], in_=bias_d[:].rearrange("(a c) h -> a c h", a=1)
            )
            ones_t = cp.tile([1, P], b16)
            nc.vector.memset(ones_t[:], 1.0)
            ident_t = cp.tile([P, P], b16)
            make_identity(nc, ident_t[:])

            # own-node activations stay resident in SBUF across layers
            # (h' = h + 1, bf16), updated in place block by block
            h_sb = cp.tile([P, NB, H], b16)
            nc.sync.dma_start(
                out=h_sb[:], in_=xown_d[:].rearrange("(k p) h -> p k h", p=P))

            # internal DRAM for inter-layer activations (AllGather path)
            addr_space = "Shared" if CC_SHARED_ else "Local"
            h_tab = [
                dr.tile([STRIDE, H], tabdt, tag=f"h_tab{i}", name=f"h_tab{i}")
                for i in range(2)
            ]
            h_full = [
                dr.tile([TAB, H], tabdt, tag=f"h_full{i}", name=f"h_full{i}",
                        addr_space=addr_space)
                for i in range(2)
            ]

            for layer in range(N_LAYERS_):
                last = layer == N_LAYERS_ - 1
                if layer == 0:
                    src_tab = xtab_d
                else:
                    src_tab = h_full[layer - 1] if USE_CC_ else xtab_d
                tab_lo = src_tab[:HI_BASE, :]
                tab_hi = src_tab[HI_BASE:, :]

                for g in range(NG):
                    blocks = _group_blocks(g)
                    ng = len(blocks)
                    nsub = ng * st
                    nlo = ng * sub_lo
                    nhi = ng * sub_hi

                    # ---- per-group gather indices ----
                    idx_t = idxp.tile([P, gw * 8], mybir.dt.int16, tag="idx")
                    nc.scalar.dma_start(
                        out=idx_t[:, 0:nsub * 8],
                        in_=idx_d[:, g * gw * 8:(g * gw + nsub) * 8])

                    # ---- gather messages (lo/hi into separate tiles) ----
                    m_lo = mlo.tile([P, SL, H], tabdt, tag="mlo")
                    m_hi = mhi.tile([P, SH, H], tabdt, tag="mhi")
                    for tab, mt, s0, s1 in ((tab_lo, m_lo, 0, nlo),
                                            (tab_hi, m_hi, nlo, nsub)):
                        step = (s1 - s0) if GMAX_ <= 0 else GMAX_
                        for g0 in range(s0, s1, step):
                            g1 = min(g0 + step, s1)
                            nc.gpsimd.dma_gather(
                                mt[:, g0 - s0:g1 - s0, :], tab,
                                idx_t[:, g0 * 8:g1 * 8],
                                (g1 - g0) * P, (g1 - g0) * P, H,
                                single_packet=True,
                            )

                    # ---- one-hot: resident, streamed, or DVE-built ----
                    if HOST_OH_ and OH_RES_:
                        oh_lo_src, oh_lo_base = ohlo_t, g * SL * P
                        oh_hi_src, oh_hi_base = ohhi_t, g * SH * P
                    elif HOST_OH_:
                        oh_lo_src = ohp.tile([P, gw * P], ohdt, tag="onehot")
                        nc.scalar.dma_start(
                            out=oh_lo_src[:, 0:nlo * P],
                            in_=ohlo_d[:, g * SL * P:(g * SL + nlo) * P],
                        )
                        nc.scalar.dma_start(
                            out=oh_lo_src[:, nlo * P:nsub * P],
                            in_=ohhi_d[:, g * SH * P:(g * SH + nhi) * P],
                        )
                        oh_hi_src = oh_lo_src
                        oh_lo_base, oh_hi_base = 0, nlo * P
                    else:
                        oh_lo_src = ohp.tile([P, gw * P], b16, tag="onehot")
                        nc.vector.tensor_tensor(
                            out=oh_lo_src[:, 0:nsub * P].rearrange(
                                "p (k s) -> p k s", k=nsub),
                            in0=seg_t[:, g * gw:g * gw + nsub].to_broadcast(
                                [P, nsub, P]),
                            in1=iota_t[:, 0:nsub * P].rearrange(
                                "p (k s) -> p k s", k=nsub),
                            op=mybir.AluOpType.is_equal,
                        )
                        oh_hi_src = oh_lo_src
                        oh_lo_base, oh_hi_base = 0, nlo * P

                    use_dr = DR_ and tabdt == fp8d and ohdt == fp8d
                    for i, b in enumerate(blocks):
                        # ---- segmented sum (lo subtiles, then hi) ----
                        psum_agg = pa.tile([P, H], f32, tag="pagg")
                        runs = ((oh_lo_src, oh_lo_base, m_lo,
                                 i * sub_lo, sub_lo),
                                (oh_hi_src, oh_hi_base, m_hi,
                                 i * sub_hi, sub_hi))
                        if use_dr:
                            nmm = sum(cnt // 2 + cnt % 2 for *_, cnt in runs)
                        else:
                            nmm = sub_lo + sub_hi
                        jj = 0
                        for oh_s, oh_b, mt, j0, cnt in runs:
                            k = 0
                            while k < cnt:
                                j = j0 + k
                                c0 = oh_b + j * P
                                if use_dr and k + 1 < cnt:
                                    nc.tensor.matmul(
                                        out=psum_agg[:],
                                        lhsT=oh_s[:, c0:c0 + 2 * P].rearrange(
                                            "p (t s) -> p t s", t=2),
                                        rhs=mt[:, j:j + 2, :],
                                        start=(jj == 0),
                                        stop=(jj == nmm - 1),
                                        perf_mode=mybir.MatmulPerfMode
                                        .DoubleRow,
                                    )
                                    k += 2
                                else:
                                    nc.tensor.matmul(
                                        out=psum_agg[:],
                                        lhsT=oh_s[:, c0:c0 + P],
                                        rhs=mt[:, j, :],
                                        start=(jj == 0),
                                        stop=(jj == nmm - 1),
                                    )
                                    k += 1
                                jj += 1

                        # ---- mean (1/deg) on Act engine ----
                        agg_bf = actp.tile([P, H], b16, tag="aggbf")
                        nc.scalar.activation(
                            agg_bf[:], psum_agg[:], AT.Copy,
                            scale=recip_t[:, b:b + 1],
                        )

                        # ---- transposes (agg | x), feature-major chunks ----
                        tr_ps = ptr.tile([P, 4, P], b16, tag="trps")
                        nc.tensor.transpose(out=tr_ps[:, 0, :],
                                            in_=agg_bf[:, 0:P],
                                            identity=ident_t[:])
                        nc.tensor.transpose(out=tr_ps[:, 1, :],
                                            in_=agg_bf[:, P:H],
                                            identity=ident_t[:])
                        nc.tensor.transpose(out=tr_ps[:, 2, :],
                                            in_=h_sb[:, b, 0:P],
                                            identity=ident_t[:])
                        nc.tensor.transpose(out=tr_ps[:, 3, :],
                                            in_=h_sb[:, b, P:H],
                                            identity=ident_t[:])
                        actT = actp.tile([P, 4, P], b16, tag="actT")
                        nc.vector.tensor_copy(out=actT[:], in_=tr_ps[:])

                        # ---- dense: zz = agg@Wl.T + x@Wr.T + b' ----
                        psum_out = po.tile([P, H], f32, tag="pout")
                        wb = layer * 4
                        for i4 in range(4):
                            nc.tensor.matmul(
                                out=psum_out[:],
                                lhsT=actT[:, i4, :],
                                rhs=wts_t[:, wb + i4, :],
                                start=(i4 == 0),
                                stop=False,
                            )
                        nc.tensor.matmul(
                            out=psum_out[:],
                            lhsT=ones_t[:],
                            rhs=bias_t[:, layer, :],
                            start=False,
                            stop=True,
                        )

                        # ---- shifted ELU: h' = max(zz, exp(min(zz,1)-1)) ----
                        if ACT_ELU_:
                            zz_t = elup.tile([P, H], f32 if last else b16,
                                             tag="zz")
                            nc.scalar.activation(zz_t[:], psum_out[:], AT.Copy)
                            m_t = elup.tile([P, H], b16, tag="m")
                            nc.vector.tensor_scalar(
                                out=m_t[:], in0=zz_t[:],
                                scalar1=1.0, scalar2=-1.0,
                                op0=mybir.AluOpType.min,
                                op1=mybir.AluOpType.add,
                            )
                            e_t = elup.tile([P, H], b16, tag="e")
                            nc.scalar.activation(e_t[:], m_t[:], AT.Exp)
                            if last:
                                ot = elup.tile([P, H], f32, tag="ot")
                                nc.vector.tensor_tensor(
                                    out=ot[:], in0=zz_t[:], in1=e_t[:],
                                    op=mybir.AluOpType.max,
                                )
                                oo = elup.tile([P, H], f32, tag="oo")
                                nc.vector.tensor_scalar(
                                    out=oo[:], in0=ot[:],
                                    scalar1=-1.0, scalar2=None,
                                    op0=mybir.AluOpType.add,
                                )
                                nc.sync.dma_start(
                                    out=out_d[b * P:(b + 1) * P, :],
                                    in_=oo[:])
                            else:
                                nc.vector.tensor_tensor(
                                    out=h_sb[:, b, :], in0=zz_t[:],
                                    in1=e_t[:],
                                    op=mybir.AluOpType.max,
                                )
                        else:
                            m_t = elup.tile([P, H], b16, tag="m")
                            nc.vector.tensor_scalar(
                                out=m_t[:], in0=psum_out[:],
                                scalar1=1.0, scalar2=-1.0,
                                op0=mybir.AluOpType.min,
                                op1=mybir.AluOpType.add,
                            )
                            e_t = elup.tile([P, H], f32, tag="e")
                            nc.scalar.activation(e_t[:], m_t[:], AT.Exp)
                            if last:
                                ot = elup.tile([P, H], f32, tag="ot")
                                nc.vector.tensor_tensor(
                                    out=ot[:], in0=psum_out[:], in1=e_t[:],
                                    op=mybir.AluOpType.max,
                                )
                                oo = elup.tile([P, H], f32, tag="oo")
                                nc.vector.tensor_scalar(
                                    out=oo[:], in0=ot[:],
                                    scalar1=-1.0, scalar2=None,
                                    op0=mybir.AluOpType.add,
                                )
                                nc.sync.dma_start(
                                    out=out_d[b * P:(b + 1) * P, :],
                                    in_=oo[:])
                            else:
                                nc.vector.tensor_tensor(
                                    out=h_sb[:, b, :], in0=psum_out[:],
                                    in1=e_t[:],
                                    op=mybir.AluOpType.max,
                                )

                    # ---- group-batched table write (for AllGather) ----
                    rows = slice(g * GS * P, (g * GS + ng) * P)
                    ksl = slice(g * GS, g * GS + ng)
                    if not last:
                        if TAB8_:
                            # centered table: store h = h' - 1 so fp8 error
                            # scales with |h| (small) instead of |h + 1|
                            h8_g = hgp.tile([P, GS, H], fp8d, tag="h8g")
                            nc.scalar.activation(
                                h8_g[:, 0:ng, :].rearrange("p k h -> p (k h)"),
                                h_sb[:, ksl, :].rearrange("p k h -> p (k h)"),
                                AT.Copy, bias=-1.0,
                            )
                            nc.sync.dma_start(
                                out=h_tab[layer][rows, :].rearrange(
                                    "(k p) h -> p k h", p=P),
                                in_=h8_g[:, 0:ng, :])
                        else:
                            nc.sync.dma_start(
                                out=h_tab[layer][rows, :].rearrange(
                                    "(k p) h -> p k h", p=P),
                                in_=h_sb[:, ksl, :])

                if layer < min(2, N_LAYERS_ - 1) and USE_CC_:
                    nc.gpsimd.collective_compute(
                        "AllGather",
                        mybir.AluOpType.bypass,
                        ins=[h_tab[layer][:]],
                        outs=[h_full[layer][:]],
                        replica_groups=[list(range(NC))],
                    )

    nc.compile()
    return nc


_CACHE = {}


def _get_program(sub_lo: int, sub_hi: int, shift: bool):
    key = (sub_lo, sub_hi, shift)
    if key not in _CACHE:
        _CACHE[key] = _build(sub_lo, sub_hi, shift)
    return _CACHE[key]


def _make_in_maps(inputs: dict, pp: dict) -> list:
    x = np.asarray(inputs["x"], dtype=np.float32)
    st = pp["st"]
    gw = GS * st
    table_row = pp["table_row"]
    shift = pp["shift_ok"]

    # permuted, padded table (fp8 or bf16) + bf16 own-feature shards
    xtab = np.zeros((TAB, H), dtype=f8 if XTAB8 else bf16)
    xtab[table_row] = x.astype(xtab.dtype)
    xown = np.zeros((TAB, H), dtype=bf16)
    xown[table_row] = x.astype(bf16)

    # weights: per layer [WlT chunk0, WlT chunk1, WrT chunk0, WrT chunk1]
    wchunks = []
    for l in range(3):
        for name in (f"Wl{l + 1}", f"Wr{l + 1}"):
            WT = np.asarray(inputs[name], dtype=np.float32).T.astype(bf16)
            wchunks.append(WT[0:P, :])
            wchunks.append(WT[P:H, :])
    wts = np.concatenate(wchunks, axis=0)  # [12*128, 256]

    # bias with the ELU/shift folds (device computes zz = z + 1):
    #   layer 0: b + 1
    #   layer 1,2 with centered fp8 h-table (stores h):  b - Wr.sum(1) + 1
    #     (only the x-path input h' = h + 1 needs correcting)
    #   layer 1,2 with bf16 table (stores h' = h + 1): b - Wl.sum - Wr.sum + 1
    biases = []
    for l in range(3):
        b = np.asarray(inputs[f"bl{l + 1}"], dtype=np.float32).copy()
        if l > 0:
            b -= np.asarray(inputs[f"Wr{l + 1}"], dtype=np.float32).sum(axis=1)
            if not TAB8:
                assert shift, "unshifted bf16 table path removed"
                b -= np.asarray(inputs[f"Wl{l + 1}"],
                                dtype=np.float32).sum(axis=1)
        b += 1.0
        biases.append(b)
    bias = np.stack(biases).astype(bf16)

    in_maps = []
    for c in range(NC):
        m = {
            "xtab": xtab,
            "xown": xown[c * STRIDE:(c + 1) * STRIDE],
            "wts": wts,
            "bias": bias,
            "idxall": pp["idx_all"][c],
            "recipall": pp["recip_all"][c],
        }
        if HOST_OH:
            m["ohloall"] = pp["oh_lo_all"][c]
            m["ohhiall"] = pp["oh_hi_all"][c]
        else:
            m["segall"] = pp["seg_all"][c]
            m["iotarep"] = np.tile(
                np.arange(P, dtype=np.float32), (P, gw)).astype(bf16)
        in_maps.append(m)
    return in_maps


def run(inputs: dict, trace: bool = False):
    """Returns (output [N_NODES, H] float32, exec_time_ns or None)."""
    edge_index = np.asarray(inputs["edge_index"])
    pp = _preprocess(edge_index)
    table_row = pp["table_row"]
    in_maps = _make_in_maps(inputs, pp)
    nc = _get_program(pp["sub_lo"], pp["sub_hi"], pp["shift_ok"])

    res = run_bass_kernel_spmd(nc, in_maps, core_ids=list(range(NC)),
                               trace=trace)

    out_full = np.empty((N_NODES, H), dtype=np.float32)
    for c in range(NC):
        shard = res.results[c]["out"]  # [STRIDE, H]
        rows = table_row - c * STRIDE
        mask = (rows >= 0) & (rows < STRIDE)
        out_full[mask] = shard[rows[mask]]
    return out_full, res.exec_time_ns


def kernel(**inputs) -> np.ndarray:
    out, _ = run(inputs)
    return out


# revision 49
# speedup vs baseline: 1.8156x; 1.1352x over previous
"""3-layer GraphSAGE (mean aggregation) on 8 TRN2 NeuronCores.

Self-contained: hardcoded problem shapes (N=50000, E=800000, H=256, 3 layers).

Strategy
--------
Host side (numpy, inside kernel()):
  * degree-balanced assignment of nodes to 8 cores x 49 blocks of <=128 nodes
  * permuted "table" layout: table row = core*6272 + block*128 + pos
  * blocks processed in GROUPS of GS=7 (uniform 7 groups); per-group edge
    lists as int16 gather indices (lo/hi split at row 32768 for int16 range),
    padded to uniform subtile counts so all 8 cores run one SPMD program
  * the segment-sum one-hot matrix (edge -> within-block position) is
    layer-invariant, so it is built on the HOST in fp8 (0/1 exact) and kept
    RESIDENT in SBUF for all three layers
  * tables gathered from are fp8; inter-layer tables store CENTERED h
    (= h' - 1 = raw ELU output, small magnitudes) so fp8 quantization error
    shrinks ~2x; the +1 shifts / Wr-sum corrections fold into the biases
  * ELU computed as max(z+1, exp(min(z+1,1)-1)) - 1

Device side (Bass/Tile), per layer:
  gather fp8 messages (8-subtile calls; bigger SWDGE rings crash HW) ->
  segmented sum via fp8 DoubleRow TensorE matmuls (2 subtiles/instruction)
  against the resident one-hot -> 1/deg scale on Activation engine ->
  PE transposes -> dense bf16 matmuls against W^T (+ K=1 bias matmul) ->
  shifted ELU (Act copy/exp + DVE min/max on bf16) -> h' stays resident in
  SBUF (in-place per block); per group a centered fp8 copy is written to
  DRAM for the AllGather (Shared output) between layers.
"""

import os
import sys

sys.path.insert(0, "/opt/trn_rl_repo")

import numpy as np
import ml_dtypes

from concourse import bacc, bass, mybir, tile
from concourse.bass_utils import run_bass_kernel_spmd
from concourse.masks import make_identity

bf16 = ml_dtypes.bfloat16
f8 = ml_dtypes.float8_e4m3

N_NODES = 50000
N_EDGES = 800000
H = 256
NC = 8
P = 128
NB = 49                      # blocks per core
HI_BASE = 32768              # int16 index split point

# knobs (env-overridable for experiments; defaults are the shipping config)
GS = int(os.environ.get("GCN_GS", "7"))          # blocks per gather group
TAB8 = os.environ.get("GCN_TAB8", "1") == "1"    # fp8 h-tables + messages
XTAB8 = os.environ.get("GCN_XTAB8",
                       os.environ.get("GCN_TAB8", "1")) == "1"  # fp8 x table
OH8 = os.environ.get("GCN_OH8", "1") == "1"      # fp8 one-hot (0/1 exact)
HOST_OH = os.environ.get("GCN_HOST_OH", "1") == "1"  # host-built one-hot
OH_RES = os.environ.get("GCN_OH_RES", "auto")    # SBUF-resident lo one-hot:
# "auto" = only when messages are fp8 (bf16 messages + resident oh overflow)
ACT_ELU = os.environ.get("GCN_ACT_ELU", "1") == "1"  # ELU via Act engine
DR = os.environ.get("GCN_DR", "1") == "1"        # DoubleRow on fp8 layers
CC_SHARED = os.environ.get("GCN_CC_SHARED", "1") == "1"
GMAX = int(os.environ.get("GCN_GMAX", "8"))      # subtiles per gather call
DMA_SCRATCH = int(os.environ.get("GCN_SCRATCH", "16384"))  # SWDGE ring bytes
# NOTE: dynamic_dma_scratch_size is charged PER PARTITION in SBUF, and rings
# bigger than the default 16384 (1024-desc) / gather calls above 1024 indices
# crash real HW (NRT_EXEC_UNIT_UNRECOVERABLE) — keep 16384/GMAX=8.
N_LAYERS = int(os.environ.get("GCN_LAYERS", "3"))
USE_CC = os.environ.get("GCN_CC", "1") == "1"

NG = (NB + GS - 1) // GS     # groups per core
STRIDE = NB * P              # 6272 table rows per core
TAB = NC * STRIDE            # 50176 table rows


def _group_blocks(g: int) -> list:
    return list(range(g * GS, min((g + 1) * GS, NB)))


def _assign_blocks(deg: np.ndarray) -> np.ndarray:
    """Serpentine deal of nodes (sorted by degree desc) into NC*NB blocks."""
    nb_total = NC * NB
    order = np.argsort(-deg, kind="stable")
    block_of_node = np.empty(N_NODES, dtype=np.int64)
    pos = 0
    rnd = 0
    while pos < N_NODES:
        take = min(nb_total, N_NODES - pos)
        blocks = np.arange(nb_total) if rnd % 2 == 0 else np.arange(nb_total)[::-1]
        block_of_node[order[pos:pos + take]] = blocks[:take]
        pos += take
        rnd += 1
    return block_of_node


def _preprocess(edge_index: np.ndarray):
    """Graph preprocessing. Returns dict of host-side structures."""
    src = np.asarray(edge_index[0], dtype=np.int64)
    dst = np.asarray(edge_index[1], dtype=np.int64)
    deg = np.bincount(dst, minlength=N_NODES).astype(np.int64)
    shift_ok = bool(deg.min() >= 1)

    block_of_node = _assign_blocks(deg)

    # position of each node within its block; table row of each node
    order = np.lexsort((np.arange(N_NODES), block_of_node))
    pos_in_block = np.empty(N_NODES, dtype=np.int64)
    counts = np.zeros(NC * NB, dtype=np.int64)
    for n in order:
        b = block_of_node[n]
        pos_in_block[n] = counts[b]
        counts[b] += 1
    assert counts.max() <= P, f"block overflow: {counts.max()}"
    table_row = block_of_node * P + pos_in_block

    # edges grouped by destination block
    e_block = block_of_node[dst]
    e_seg = pos_in_block[dst]
    e_srcrow = table_row[src]

    sort_idx = np.argsort(e_block, kind="stable")
    e_block_s = e_block[sort_idx]
    e_seg_s = e_seg[sort_idx]
    e_srcrow_s = e_srcrow[sort_idx]
    blk_starts = np.searchsorted(e_block_s, np.arange(NC * NB + 1))

    lo_counts = np.empty(NC * NB, dtype=np.int64)
    hi_counts = np.empty(NC * NB, dtype=np.int64)
    for b in range(NC * NB):
        rows = e_srcrow_s[blk_starts[b]:blk_starts[b + 1]]
        lo_counts[b] = int((rows < HI_BASE).sum())
        hi_counts[b] = rows.shape[0] - lo_counts[b]
    sub_lo = int(np.ceil(lo_counts.max() / P))
    sub_hi = int(np.ceil(hi_counts.max() / P))
    st = sub_lo + sub_hi

    # per-core packed arrays, group layout:
    #   subtile order per group: [lo(b0)..lo(bN)][hi(b0)..hi(bN)]
    gw = GS * st                                  # subtiles per (full) group
    idx_all = np.zeros((NC, P, NG * gw * 8), dtype=np.int16)
    seg_all = np.full((NC, P, NG * gw), 200.0, dtype=np.float32)
    recip_all = np.zeros((NC, P, NB), dtype=np.float32)

    recip = (1.0 / np.maximum(deg, 1)).astype(np.float32)

    def pack16(flat: np.ndarray) -> np.ndarray:
        # dma_gather layout: unwrapped[k] = tile16[k % 16, k // 16]
        n = flat.shape[0]
        t = flat.reshape(n // 16, 16).T
        return np.tile(t, (8, 1))  # [128, n/16]

    def padded(rows, segs, nsub):
        r = np.zeros(nsub * P, dtype=np.int16)
        r[:rows.shape[0]] = rows.astype(np.int16)
        s = np.full(nsub * P, 200.0, dtype=np.float32)
        s[:segs.shape[0]] = segs.astype(np.float32)
        return r, s.reshape(nsub, P).T  # seg -> [P, nsub]

    for c in range(NC):
        for g in range(NG):
            blocks = _group_blocks(g)
            los, his = [], []
            for lb in blocks:
                b = c * NB + lb
                rows = e_srcrow_s[blk_starts[b]:blk_starts[b + 1]]
                segs = e_seg_s[blk_starts[b]:blk_starts[b + 1]]
                is_lo = rows < HI_BASE
                los.append(padded(rows[is_lo], segs[is_lo], sub_lo))
                his.append(padded(rows[~is_lo] - HI_BASE, segs[~is_lo], sub_hi))

            ng = len(blocks)
            ibase = g * gw * 8
            sbase = g * gw
            lo_flat = np.concatenate([r for r, _ in los])
            hi_flat = np.concatenate([r for r, _ in his])
            idx_all[c, :, ibase:ibase + ng * sub_lo * 8] = pack16(lo_flat)
            idx_all[c, :, ibase + ng * sub_lo * 8:
                    ibase + ng * st * 8] = pack16(hi_flat)
            seg_all[c, :, sbase:sbase + ng * sub_lo] = np.concatenate(
                [s for _, s in los], axis=1)
            seg_all[c, :, sbase + ng * sub_lo:sbase + ng * st] = np.concatenate(
                [s for _, s in his], axis=1)

            for lb in blocks:
                b = c * NB + lb
                nodes_here = np.where(block_of_node == b)[0]
                recip_all[c, pos_in_block[nodes_here], lb] = recip[nodes_here]

    out = dict(
        table_row=table_row, sub_lo=sub_lo, sub_hi=sub_hi, st=st,
        idx_all=idx_all, seg_all=seg_all.astype(bf16), recip_all=recip_all,
        shift_ok=shift_ok,
    )

    if HOST_OH:
        # host-built one-hot: oh[c, p, j, s] = (seg(edge p of subtile j) == s)
        # in fp8 (0/1 exact), split into lo/hi subtile parts per group
        oh = (seg_all[..., None] == np.arange(P, dtype=np.float32)) \
            .astype(f8 if OH8 else bf16)
        oh = oh.reshape(NC, P, NG, gw, P)
        SL, SH = GS * sub_lo, GS * sub_hi
        oh_lo = np.zeros((NC, P, NG, SL, P), dtype=oh.dtype)
        oh_hi = np.zeros((NC, P, NG, SH, P), dtype=oh.dtype)
        for g in range(NG):
            ng = len(_group_blocks(g))
            nlo, nhi = ng * sub_lo, ng * sub_hi
            oh_lo[:, :, g, :nlo] = oh[:, :, g, :nlo]
            oh_hi[:, :, g, :nhi] = oh[:, :, g, nlo:nlo + nhi]
        out["oh_lo_all"] = oh_lo.reshape(NC, P, NG * SL * P)
        out["oh_hi_all"] = oh_hi.reshape(NC, P, NG * SH * P)
    return out


def _group_subtiles(i: int, ng: int, sub_lo: int, sub_hi: int) -> list:
    """Subtile columns of block i (0-based within group) in a group of ng."""
    lo = list(range(i * sub_lo, (i + 1) * sub_lo))
    hi = [ng * sub_lo + i * sub_hi + j for j in range(sub_hi)]
    return lo + hi


def _build(sub_lo: int, sub_hi: int, shift: bool,
           use_cc: bool | None = None,
           cc_shared: bool | None = None, n_layers: int | None = None,
           gmax: int | None = None, scratch: int | None = None,
           tab8: bool | None = None, xtab8: bool | None = None,
           oh8: bool | None = None, dr: bool | None = None,
           host_oh: bool | None = None,
           oh_res: bool | None = None, act_elu: bool | None = None):
    """Build the SPMD Bass program. Returns compiled nc."""
    USE_CC_ = USE_CC if use_cc is None else use_cc
    CC_SHARED_ = CC_SHARED if cc_shared is None else cc_shared
    N_LAYERS_ = N_LAYERS if n_layers is None else n_layers
    GMAX_ = GMAX if gmax is None else gmax
    SCRATCH_ = DMA_SCRATCH if scratch is None else scratch
    TAB8_ = TAB8 if tab8 is None else tab8
    XTAB8_ = XTAB8 if xtab8 is None else xtab8
    OH8_ = OH8 if oh8 is None else oh8
    DR_ = (DR if dr is None else dr) and OH8_
    HOST_OH_ = HOST_OH if host_oh is None else host_oh
    if oh_res is None:
        oh_res = (TAB8_ and XTAB8_) if OH_RES == "auto" else OH_RES == "1"
    OH_RES_ = oh_res and HOST_OH_
    ACT_ELU_ = ACT_ELU if act_elu is None else act_elu

    st = sub_lo + sub_hi
    gw = GS * st
    SL = GS * sub_lo            # lo subtiles per full group
    SH = GS * sub_hi            # hi subtiles per full group
    f32 = mybir.dt.float32
    b16 = mybir.dt.bfloat16
    fp8d = mybir.dt.float8e4
    xdt = fp8d if XTAB8_ else b16    # layer-0 table (x) dtype
    hdt = fp8d if TAB8_ else b16     # inter-layer table (h) dtype
    ohdt = fp8d if OH8_ else b16
    assert xdt == hdt, "per-layer message dtypes not supported yet"
    tabdt = xdt
    AT = mybir.ActivationFunctionType

    nc = bacc.Bacc("TRN2", target_bir_lowering=False, debug=False,
                   enable_asserts=True, num_devices=NC,
                   dynamic_dma_scratch_size=SCRATCH_)

    xtab_d = nc.dram_tensor("xtab", [TAB, H], tabdt, kind="ExternalInput")
    xown_d = nc.dram_tensor("xown", [STRIDE, H], b16, kind="ExternalInput")
    wts_d = nc.dram_tensor("wts", [12 * P, H], b16, kind="ExternalInput")
    bias_d = nc.dram_tensor("bias", [3, H], b16, kind="ExternalInput")
    idx_d = nc.dram_tensor("idxall", [P, NG * gw * 8], mybir.dt.int16,
                           kind="ExternalInput")
    recip_d = nc.dram_tensor("recipall", [P, NB], f32, kind="ExternalInput")
    if HOST_OH_:
        ohlo_d = nc.dram_tensor("ohloall", [P, NG * SL * P], ohdt,
                                kind="ExternalInput")
        ohhi_d = nc.dram_tensor("ohhiall", [P, NG * SH * P], ohdt,
                                kind="ExternalInput")
    else:
        seg_d = nc.dram_tensor("segall", [P, NG * gw], b16,
                               kind="ExternalInput")
        iota_d = nc.dram_tensor("iotarep", [P, gw * P], b16,
                                kind="ExternalInput")

    out_d = nc.dram_tensor("out", [STRIDE, H], f32, kind="ExternalOutput")

    with tile.TileContext(nc) as tc:
        with (
            tc.tile_pool(name="const", bufs=1) as cp,
            tc.tile_pool(name="mlo", bufs=2) as mlo,
            tc.tile_pool(name="mhi", bufs=1 if OH_RES_ else 2) as mhi,
            tc.tile_pool(name="ohp", bufs=2) as ohp,
            tc.tile_pool(name="idxp", bufs=2) as idxp,
            tc.tile_pool(name="hgp", bufs=2) as hgp,
            tc.tile_pool(name="actp", bufs=3) as actp,
            tc.tile_pool(name="elup", bufs=2) as elup,
            tc.tile_pool(name="pa", bufs=4, space="PSUM") as pa,
            tc.tile_pool(name="po", bufs=2, space="PSUM") as po,
            tc.tile_pool(name="ptr", bufs=2, space="PSUM") as ptr,
            tc.tile_pool(name="dram", bufs=1, space="DRAM") as dr,
        ):
            # ---- resident constants ----
            recip_t = cp.tile([P, NB], f32)
            nc.sync.dma_start(out=recip_t[:], in_=recip_d[:])
            if HOST_OH_ and OH_RES_:
                # SBUF-resident one-hot (layer-invariant), loaded once.
                ohlo_t = cp.tile([P, NG * SL * P], ohdt)
                ohhi_t = cp.tile([P, NG * SH * P], ohdt)
                for g in range(NG):
                    nc.scalar.dma_start(
                        out=ohlo_t[:, g * SL * P:(g + 1) * SL * P],
                        in_=ohlo_d[:, g * SL * P:(g + 1) * SL * P])
                    nc.scalar.dma_start(
                        out=ohhi_t[:, g * SH * P:(g + 1) * SH * P],
                        in_=ohhi_d[:, g * SH * P:(g + 1) * SH * P])
            if not HOST_OH_:
                seg_t = cp.tile([P, NG * gw], b16)
                nc.sync.dma_start(out=seg_t[:], in_=seg_d[:])
                iota_t = cp.tile([P, gw * P], b16)
                nc.sync.dma_start(out=iota_t[:], in_=iota_d[:])
            wts_t = cp.tile([P, 12, H], b16)
            nc.sync.dma_start(
                out=wts_t[:], in_=wts_d[:].rearrange("(c k) h -> k c h", k=P)
            )
            bias_t = cp.tile([1, 3, H], b16)
            nc.sync.dma_start(
                out=bias_t[:], in_=bias_d[:].rearrange("(a c) h -> a c h", a=1)
            )
            ones_t = cp.tile([1, P], b16)
            nc.vector.memset(ones_t[:], 1.0)
            ident_t = cp.tile([P, P], b16)
            make_identity(nc, ident_t[:])

            # own-node activations stay resident in SBUF across layers
            # (h' = h + 1, bf16), updated in place block by block
            h_sb = cp.tile([P, NB, H], b16)
            nc.sync.dma_start(
                out=h_sb[:], in_=xown_d[:].rearrange("(k p) h -> p k h", p=P))

            # internal DRAM for inter-layer activations (AllGather path)
            addr_space = "Shared" if CC_SHARED_ else "Local"
            h_tab = [
                dr.tile([STRIDE, H], tabdt, tag=f"h_tab{i}", name=f"h_tab{i}")
                for i in range(2)
            ]
            h_full = [
                dr.tile([TAB, H], tabdt, tag=f"h_full{i}", name=f"h_full{i}",
                        addr_space=addr_space)
                for i in range(2)
            ]

            for layer in range(N_LAYERS_):
                last = layer == N_LAYERS_ - 1
                if layer == 0:
                    src_tab = xtab_d
                else:
                    src_tab = h_full[layer - 1] if USE_CC_ else xtab_d
                tab_lo = src_tab[:HI_BASE, :]
                tab_hi = src_tab[HI_BASE:, :]

                for g in range(NG):
                    blocks = _group_blocks(g)
                    ng = len(blocks)
                    nsub = ng * st
                    nlo = ng * sub_lo
                    nhi = ng * sub_hi

                    # ---- per-group gather indices ----
                    idx_t = idxp.tile([P, gw * 8], mybir.dt.int16, tag="idx")
                    nc.scalar.dma_start(
                        out=idx_t[:, 0:nsub * 8],
                        in_=idx_d[:, g * gw * 8:(g * gw + nsub) * 8])

                    # ---- gather messages (lo/hi into separate tiles) ----
                    m_lo = mlo.tile([P, SL, H], tabdt, tag="mlo")
                    m_hi = mhi.tile([P, SH, H], tabdt, tag="mhi")
                    for tab, mt, s0, s1 in ((tab_lo, m_lo, 0, nlo),
                                            (tab_hi, m_hi, nlo, nsub)):
                        step = (s1 - s0) if GMAX_ <= 0 else GMAX_
                        for g0 in range(s0, s1, step):
                            g1 = min(g0 + step, s1)
                            nc.gpsimd.dma_gather(
                                mt[:, g0 - s0:g1 - s0, :], tab,
                                idx_t[:, g0 * 8:g1 * 8],
                                (g1 - g0) * P, (g1 - g0) * P, H,
                                single_packet=True,
                            )

                    # ---- one-hot: resident, streamed, or DVE-built ----
                    if HOST_OH_ and OH_RES_:
                        oh_lo_src, oh_lo_base = ohlo_t, g * SL * P
                        oh_hi_src, oh_hi_base = ohhi_t, g * SH * P
                    elif HOST_OH_:
                        oh_lo_src = ohp.tile([P, gw * P], ohdt, tag="onehot")
                        nc.scalar.dma_start(
                            out=oh_lo_src[:, 0:nlo * P],
                            in_=ohlo_d[:, g * SL * P:(g * SL + nlo) * P],
                        )
                        nc.scalar.dma_start(
                            out=oh_lo_src[:, nlo * P:nsub * P],
                            in_=ohhi_d[:, g * SH * P:(g * SH + nhi) * P],
                        )
                        oh_hi_src = oh_lo_src
                        oh_lo_base, oh_hi_base = 0, nlo * P
                    else:
                        oh_lo_src = ohp.tile([P, gw * P], b16, tag="onehot")
                        nc.vector.tensor_tensor(
                            out=oh_lo_src[:, 0:nsub * P].rearrange(
                                "p (k s) -> p k s", k=nsub),
                            in0=seg_t[:, g * gw:g * gw + nsub].to_broadcast(
                                [P, nsub, P]),
                            in1=iota_t[:, 0:nsub * P].rearrange(
                                "p (k s) -> p k s", k=nsub),
                            op=mybir.AluOpType.is_equal,
                        )
                        oh_hi_src = oh_lo_src
                        oh_lo_base, oh_hi_base = 0, nlo * P

                    use_dr = DR_ and tabdt == fp8d and ohdt == fp8d
                    for i, b in enumerate(blocks):
                        # ---- segmented sum (lo subtiles, then hi) ----
                        psum_agg = pa.tile([P, H], f32, tag="pagg")
                        runs = ((oh_lo_src, oh_lo_base, m_lo,
                                 i * sub_lo, sub_lo),
                                (oh_hi_src, oh_hi_base, m_hi,
                                 i * sub_hi, sub_hi))
                        if use_dr:
                            nmm = sum(cnt // 2 + cnt % 2 for *_, cnt in runs)
                        else:
                            nmm = sub_lo + sub_hi
                        jj = 0
                        for oh_s, oh_b, mt, j0, cnt in runs:
                            k = 0
                            while k < cnt:
                                j = j0 + k
                                c0 = oh_b + j * P
                                if use_dr and k + 1 < cnt:
                                    nc.tensor.matmul(
                                        out=psum_agg[:],
                                        lhsT=oh_s[:, c0:c0 + 2 * P].rearrange(
                                            "p (t s) -> p t s", t=2),
                                        rhs=mt[:, j:j + 2, :],
                                        start=(jj == 0),
                                        stop=(jj == nmm - 1),
                                        perf_mode=mybir.MatmulPerfMode
                                        .DoubleRow,
                                    )
                                    k += 2
                                else:
                                    nc.tensor.matmul(
                                        out=psum_agg[:],
                                        lhsT=oh_s[:, c0:c0 + P],
                                        rhs=mt[:, j, :],
                                        start=(jj == 0),
                                        stop=(jj == nmm - 1),
                                    )
                                    k += 1
                                jj += 1

                        # ---- mean (1/deg) on Act engine ----
                        agg_bf = actp.tile([P, H], b16, tag="aggbf")
                        nc.scalar.activation(
                            agg_bf[:], psum_agg[:], AT.Copy,
                            scale=recip_t[:, b:b + 1],
                        )

                        # ---- transposes (agg | x), feature-major chunks ----
                        tr_ps = ptr.tile([P, 4, P], b16, tag="trps")
                        nc.tensor.transpose(out=tr_ps[:, 0, :],
                                            in_=agg_bf[:, 0:P],
                                            identity=ident_t[:])
                        nc.tensor.transpose(out=tr_ps[:, 1, :],
                                            in_=agg_bf[:, P:H],
                                            identity=ident_t[:])
                        nc.tensor.transpose(out=tr_ps[:, 2, :],
                                            in_=h_sb[:, b, 0:P],
                                            identity=ident_t[:])
                        nc.tensor.transpose(out=tr_ps[:, 3, :],
                                            in_=h_sb[:, b, P:H],
                                            identity=ident_t[:])
                        actT = actp.tile([P, 4, P], b16, tag="actT")
                        nc.vector.tensor_copy(out=actT[:], in_=tr_ps[:])

                        # ---- dense: zz = agg@Wl.T + x@Wr.T + b' ----
                        psum_out = po.tile([P, H], f32, tag="pout")
                        wb = layer * 4
                        for i4 in range(4):
                            nc.tensor.matmul(
                                out=psum_out[:],
                                lhsT=actT[:, i4, :],
                                rhs=wts_t[:, wb + i4, :],
                                start=(i4 == 0),
                                stop=False,
                            )
                        nc.tensor.matmul(
                            out=psum_out[:],
                            lhsT=ones_t[:],
                            rhs=bias_t[:, layer, :],
                            start=False,
                            stop=True,
                        )

                        # ---- shifted ELU: h' = max(zz, exp(min(zz,1)-1)) ----
                        if ACT_ELU_:
                            zz_t = elup.tile([P, H], f32 if last else b16,
                                             tag="zz")
                            nc.scalar.activation(zz_t[:], psum_out[:], AT.Copy)
                            m_t = elup.tile([P, H], b16, tag="m")
                            nc.vector.tensor_scalar(
                                out=m_t[:], in0=zz_t[:],
                                scalar1=1.0, scalar2=-1.0,
                                op0=mybir.AluOpType.min,
                                op1=mybir.AluOpType.add,
                            )
                            e_t = elup.tile([P, H], b16, tag="e")
                            nc.scalar.activation(e_t[:], m_t[:], AT.Exp)
                            if last:
                                ot = elup.tile([P, H], f32, tag="ot")
                                nc.vector.tensor_tensor(
                                    out=ot[:], in0=zz_t[:], in1=e_t[:],
                                    op=mybir.AluOpType.max,
                                )
                                oo = elup.tile([P, H], f32, tag="oo")
                                nc.vector.tensor_scalar(
                                    out=oo[:], in0=ot[:],
                                    scalar1=-1.0, scalar2=None,
                                    op0=mybir.AluOpType.add,
                                )
                                nc.sync.dma_start(
                                    out=out_d[b * P:(b + 1) * P, :],
                                    in_=oo[:])
                            else:
                                nc.vector.tensor_tensor(
                                    out=h_sb[:, b, :], in0=zz_t[:],
                                    in1=e_t[:],
                                    op=mybir.AluOpType.max,
                                )
                        else:
                            m_t = elup.tile([P, H], b16, tag="m")
                            nc.vector.tensor_scalar(
                                out=m_t[:], in0=psum_out[:],
                                scalar1=1.0, scalar2=-1.0,
                                op0=mybir.AluOpType.min,
                                op1=mybir.AluOpType.add,
                            )
                            e_t = elup.tile([P, H], f32, tag="e")
                            nc.scalar.activation(e_t[:], m_t[:], AT.Exp)
                            if last:
                                ot = elup.tile([P, H], f32, tag="ot")
                                nc.vector.tensor_tensor(
                                    out=ot[:], in0=psum_out[:], in1=e_t[:],
                                    op=mybir.AluOpType.max,
                                )
                                oo = elup.tile([P, H], f32, tag="oo")
                                nc.vector.tensor_scalar(
                                    out=oo[:], in0=ot[:],
                                    scalar1=-1.0, scalar2=None,
                                    op0=mybir.AluOpType.add,
                                )
                                nc.sync.dma_start(
                                    out=out_d[b * P:(b + 1) * P, :],
                                    in_=oo[:])
                            else:
                                nc.vector.tensor_tensor(
                                    out=h_sb[:, b, :], in0=psum_out[:],
                                    in1=e_t[:],
                                    op=mybir.AluOpType.max,
                                )

                    # ---- group-batched table write (for AllGather) ----
                    rows = slice(g * GS * P, (g * GS + ng) * P)
                    ksl = slice(g * GS, g * GS + ng)
                    if not last:
                        if TAB8_:
                            # centered table: store h = h' - 1 so fp8 error
                            # scales with |h| (small) instead of |h + 1|
                            h8_g = hgp.tile([P, GS, H], fp8d, tag="h8g")
                            nc.scalar.activation(
                                h8_g[:, 0:ng, :].rearrange("p k h -> p (k h)"),
                                h_sb[:, ksl, :].rearrange("p k h -> p (k h)"),
                                AT.Copy, bias=-1.0,
                            )
                            nc.sync.dma_start(
                                out=h_tab[layer][rows, :].rearrange(
                                    "(k p) h -> p k h", p=P),
                                in_=h8_g[:, 0:ng, :])
                        else:
                            nc.sync.dma_start(
                                out=h_tab[layer][rows, :].rearrange(
                                    "(k p) h -> p k h", p=P),
                                in_=h_sb[:, ksl, :])

                if layer < min(2, N_LAYERS_ - 1) and USE_CC_:
                    nc.gpsimd.collective_compute(
                        "AllGather",
                        mybir.AluOpType.bypass,
                        ins=[h_tab[layer][:]],
                        outs=[h_full[layer][:]],
                        replica_groups=[list(range(NC))],
                    )

    nc.compile()
    return nc


_CACHE = {}


def _get_program(sub_lo: int, sub_hi: int, shift: bool):
    key = (sub_lo, sub_hi, shift)
    if key not in _CACHE:
        _CACHE[key] = _build(sub_lo, sub_hi, shift)
    return _CACHE[key]


def _make_in_maps(inputs: dict, pp: dict) -> list:
    x = np.asarray(inputs["x"], dtype=np.float32)
    st = pp["st"]
    gw = GS * st
    table_row = pp["table_row"]
    shift = pp["shift_ok"]

    # permuted, padded table (fp8 or bf16) + bf16 own-feature shards
    xtab = np.zeros((TAB, H), dtype=f8 if XTAB8 else bf16)
    xtab[table_row] = x.astype(xtab.dtype)
    xown = np.zeros((TAB, H), dtype=bf16)
    xown[table_row] = x.astype(bf16)

    # weights: per layer [WlT chunk0, WlT chunk1, WrT chunk0, WrT chunk1]
    wchunks = []
    for l in range(3):
        for name in (f"Wl{l + 1}", f"Wr{l + 1}"):
            WT = np.asarray(inputs[name], dtype=np.float32).T.astype(bf16)
            wchunks.append(WT[0:P, :])
            wchunks.append(WT[P:H, :])
    wts = np.concatenate(wchunks, axis=0)  # [12*128, 256]

    # bias with the ELU/shift folds (device computes zz = z + 1):
    #   layer 0: b + 1
    #   layer 1,2 with centered fp8 h-table (stores h):  b - Wr.sum(1) + 1
    #     (only the x-path input h' = h + 1 needs correcting)
    #   layer 1,2 with bf16 table (stores h' = h + 1): b - Wl.sum - Wr.sum + 1
    biases = []
    for l in range(3):
        b = np.asarray(inputs[f"bl{l + 1}"], dtype=np.float32).copy()
        if l > 0:
            b -= np.asarray(inputs[f"Wr{l + 1}"], dtype=np.float32).sum(axis=1)
            if not TAB8:
                assert shift, "unshifted bf16 table path removed"
                b -= np.asarray(inputs[f"Wl{l + 1}"],
                                dtype=np.float32).sum(axis=1)
        b += 1.0
        biases.append(b)
    bias = np.stack(biases).astype(bf16)

    in_maps = []
    for c in range(NC):
        m = {
            "xtab": xtab,
            "xown": xown[c * STRIDE:(c + 1) * STRIDE],
            "wts": wts,
            "bias": bias,
            "idxall": pp["idx_all"][c],
            "recipall": pp["recip_all"][c],
        }
        if HOST_OH:
            m["ohloall"] = pp["oh_lo_all"][c]
            m["ohhiall"] = pp["oh_hi_all"][c]
        else:
            m["segall"] = pp["seg_all"][c]
            m["iotarep"] = np.tile(
                np.arange(P, dtype=np.float32), (P, gw)).astype(bf16)
        in_maps.append(m)
    return in_maps


def run(inputs: dict, trace: bool = False):
    """Returns (output [N_NODES, H] float32, exec_time_ns or None)."""
    edge_index = np.asarray(inputs["edge_index"])
    pp = _preprocess(edge_index)
    table_row = pp["table_row"]
    in_maps = _make_in_maps(inputs, pp)
    nc = _get_program(pp["sub_lo"], pp["sub_hi"], pp["shift_ok"])

    res = run_bass_kernel_spmd(nc, in_maps, core_ids=list(range(NC)),
                               trace=trace)

    out_full = np.empty((N_NODES, H), dtype=np.float32)
    for c in range(NC):
        shard = res.results[c]["out"]  # [STRIDE, H]
        rows = table_row - c * STRIDE
        mask = (rows >= 0) & (rows < STRIDE)
        out_full[mask] = shard[rows[mask]]
    return out_full, res.exec_time_ns


def kernel(**inputs) -> np.ndarray:
    out, _ = run(inputs)
    return out
